# revision 1
# baseline (speedup 1.0000x reference)
"""AttentionWithRoPE on 8 Trainium2 NeuronCores.

Sharding: batch x query-half -> 8 independent cores (no collectives).
Core c handles batch b=c//2, query rows [qh*1024, (qh+1)*1024) with qh=c%2.
The host rolls the t axis per core so the query rows always sit in columns
[0, TQ) of xT; cosk/sink are rolled identically, so RoPE sees true positions
and the softmax key set is unchanged (order-invariant).

Per-core plan (all matmuls fp32r = full PE rate at N>=256):
  V:    V projection in natural layout [t, j], staged to DRAM per-head
        contiguous (Vd3[h] is a ready-to-load [128, 16, 65] lhsT with a
        ones column appended in SBUF).
  QK+attention, software-pipelined per head pair p:
        Q^T/K^T projections for pair p+1 (transposed layout [j, t], RoPE via
        signed pair-swap permutation matmul + DVE/GPSIMD combine) are emitted
        interleaved with attention for pair p, so the PE fills the gaps while
        ACT grinds through the exp()s of the softmax.
        Attention per head: S^T[t,l] = K^T_h.T @ Q^T_h (64-partition operands,
        head B reads partitions 64:128 directly), P^T = exp(scale*S^T) on ACT
        (no max subtraction needed; logits are O(1)), attnout^T = [V_h|1].T @
        P^T accumulated over t which also yields the softmax row-sums in
        partition 64; normalize via reciprocal + K=1 broadcast matmul issued
        from row 64, then stage normalized attnout^T to DRAM (AOTd).
  Out:  out = attnout^T.T @ Wp + bp.
"""

import sys

sys.path.insert(0, "/opt/trn_rl_repo")

from contextlib import ExitStack

import numpy as np

import concourse.bass as bass
import concourse.mybir as mybir
import concourse.tile as tile
from concourse.bass_utils import run_bass_kernel_spmd

F32 = mybir.dt.float32
F32R = mybir.dt.float32r
AF = mybir.ActivationFunctionType
MUL = mybir.AluOpType.mult

B, T, D = 4, 2048, 1024
H, HD = 16, 64
P = 128
TQ = 1024  # query rows per core
SCALE = float(D) ** -0.5
ROPE_THETA = 10000.0

_ws_ctr = [0]


def _split_multi_waits(nc):
    """The walrus build in this container accepts at most one sync-wait per
    engine instruction. Hoist all but one wait of each instruction into
    standalone EventSemaphore instructions on the same engine, placed
    immediately before it (engines are in-order, so semantics are identical)."""
    n = 0
    for f in nc.m.functions:
        for blk in f.blocks:
            insts = list(blk.instructions)
            newlist = []
            changed = False
            for inst in insts:
                si = getattr(inst, "sync_info", None)
                waits = list(si.on_wait) if si is not None and si.on_wait else []
                if len(waits) > 1:
                    for w in waits[:-1]:
                        _ws_ctr[0] += 1
                        evs = mybir.InstEventSemaphore(
                            name=f"WSPLIT-{_ws_ctr[0]}", ins=[], outs=[]
                        )
                        evs.engine = inst.engine
                        evs.sync_info = mybir.SyncInfo(on_wait=[w], on_update=[])
                        newlist.append(evs)
                        n += 1
                    inst.sync_info = mybir.SyncInfo(
                        on_wait=[waits[-1]], on_update=list(si.on_update)
                    )
                    changed = True
                newlist.append(inst)
            if changed:
                blk.instructions[:] = newlist
    return n


def _perm_lhsT():
    """lhsT for the rotate-half permutation: out = lhsT.T @ q gives
    out[2i] = -q[2i+1], out[2i+1] = q[2i]."""
    m = np.zeros((P, P), np.float32)
    for i in range(P // 2):
        m[2 * i + 1, 2 * i] = -1.0
        m[2 * i, 2 * i + 1] = 1.0
    return m


def build_nc(split_waits=True, reps=1):
    nc = bass.Bass(trn_type="TRN2", target_bir_lowering=False, debug=False)

    xT = nc.dram_tensor("xT", [D, T], F32, kind="ExternalInput").ap()
    Wq = nc.dram_tensor("Wq", [D, D], F32, kind="ExternalInput").ap()
    Wk = nc.dram_tensor("Wk", [D, D], F32, kind="ExternalInput").ap()
    Wv = nc.dram_tensor("Wv", [D, D], F32, kind="ExternalInput").ap()
    Wp = nc.dram_tensor("Wp", [D, D], F32, kind="ExternalInput").ap()
    bpb = nc.dram_tensor("bpb", [P, D], F32, kind="ExternalInput").ap()
    cosk = nc.dram_tensor("cosk", [P, T], F32, kind="ExternalInput").ap()
    sink = nc.dram_tensor("sink", [P, T], F32, kind="ExternalInput").ap()
    out = nc.dram_tensor("out", [TQ, D], F32, kind="ExternalOutput").ap()

    permc = nc.inline_tensor(_perm_lhsT(), name="permc").ap()
    ones128c = nc.inline_tensor(np.ones((P, 64), np.float32), name="ones128c").ap()
    onescolc = nc.inline_tensor(
        np.ones((P, T // P, 1), np.float32), name="onescolc"
    ).ap()

    # Vd3[h] is a contiguous [128(tp), 16(to), 65] per-head V'-block (ones
    # column filled in SBUF after load). AOTd holds normalized attnout^T.
    Vd3 = nc.dram_tensor("Vd3", [H, P, T // P, 65], F32).ap()
    AOTd = nc.dram_tensor("AOTd", [D // P, P, TQ], F32).ap()

    xT_r = xT.rearrange("(do dp) t -> dp do t", dp=P)  # [128, 8, 2048]
    Wq_r = Wq.rearrange("(do dp) j -> dp do j", dp=P)
    Wk_r = Wk.rearrange("(do dp) j -> dp do j", dp=P)
    Wv_r = Wv.rearrange("(do dp) j -> dp do j", dp=P)
    Wp_r = Wp.rearrange("(ko kp) j -> kp ko j", kp=P)
    AOTd_r = AOTd.rearrange("ko p l -> p ko l")  # [128, 8, 1024]

    DO = D // P  # 8 contraction tiles
    NT = T // P  # 16 key tiles

    with tile.TileContext(nc) as tc:
      for _rep in range(reps):
        with ExitStack() as top:
            persist = top.enter_context(tc.tile_pool(name="persist", bufs=1))
            ones128 = persist.tile([P, 64], F32R)
            onesva = persist.tile([P, NT, 1], F32R)
            permt = persist.tile([P, P], F32R)
            ck = persist.tile([P, T], F32)
            sk = persist.tile([P, T], F32)

            with tc.tile_pool(name="xpool", bufs=1) as xpool:
                xts = xpool.tile([P, DO, T], F32R)

                # -------- interleaved Q/K projection + attention pipeline ------
                with ExitStack() as pmain:
                    wv1pool = pmain.enter_context(tc.tile_pool(name="w_v1", bufs=1))
                    vpool = pmain.enter_context(tc.tile_pool(name="o_v1", bufs=2))
                    wpool = pmain.enter_context(tc.tile_pool(name="w_qk", bufs=2))
                    tpool = pmain.enter_context(tc.tile_pool(name="t_qk", bufs=2))
                    kqpool = pmain.enter_context(tc.tile_pool(name="kq", bufs=2))
                    pmm = pmain.enter_context(
                        tc.tile_pool(name="ps_qk", bufs=1, space="PSUM")
                    )
                    pmisc = pmain.enter_context(
                        tc.tile_pool(name="ps_misc", bufs=1, space="PSUM")
                    )
                    prot = pmisc
                    psb = pmisc

                    def proj_rope(w_t, dst, dst_col, src_col):
                        """One [128, 512] Q/K projection tile + RoPE into dst."""
                        ps = pmm.tile([P, 512], F32, tag="ps")
                        for do in range(DO):
                            nc.tensor.matmul(
                                ps[:],
                                w_t[:, do],
                                xts[:, do, src_col * 512 : (src_col + 1) * 512],
                                start=(do == 0),
                                stop=(do == DO - 1),
                            )
                        raw = tpool.tile([P, 512], F32R, tag="raw")
                        nc.vector.tensor_copy(raw[:], ps[:])
                        pr = pmisc.tile([P, 512], F32, tag="misc")
                        nc.tensor.matmul(pr[:], permt[:], raw[:], start=True, stop=True)
                        t1 = tpool.tile([P, 512], F32, tag="t1")
                        nc.gpsimd.tensor_tensor(
                            t1[:],
                            raw.bitcast(F32),
                            ck[:, src_col * 512 : (src_col + 1) * 512],
                            MUL,
                        )
                        t2 = tpool.tile([P, 512], F32, tag="t2")
                        nc.vector.tensor_mul(
                            t2[:], pr[:], sk[:, src_col * 512 : (src_col + 1) * 512]
                        )
                        nc.vector.tensor_add(
                            dst[:, dst_col * 512 : (dst_col + 1) * 512], t1[:], t2[:]
                        )

                    def emit_proj_jo(jo, preload=False):
                        """Q^T/K^T for head pair jo -> fresh SBUF tiles; returns
                        a list of closures (one per projection tile) plus the
                        destination tiles."""
                        ktp = kqpool.tile([P, T], F32R, tag="ktp")
                        qtp = kqpool.tile([P, TQ], F32R, tag="qtp")
                        steps = []

                        def load_w():
                            wq_t = wpool.tile([P, DO, P], F32R, tag="wq")
                            nc.gpsimd.dma_start(
                                wq_t[:], Wq_r[:, :, jo * P : (jo + 1) * P].bitcast(F32R)
                            )
                            wk_t = wpool.tile([P, DO, P], F32R, tag="wk")
                            nc.gpsimd.dma_start(
                                wk_t[:], Wk_r[:, :, jo * P : (jo + 1) * P].bitcast(F32R)
                            )
                            return wq_t, wk_t

                        wref = []
                        if preload:
                            wref.extend(load_w())

                        def step(i):
                            if i == 0 and not wref:
                                wref.extend(load_w())
                            wq_t, wk_t = wref
                            if i < 2:
                                proj_rope(wq_t, qtp, i, i)
                            else:
                                proj_rope(wk_t, ktp, i - 2, i - 2)

                        for i in range(6):
                            steps.append(lambda i=i: step(i))
                        return ktp, qtp, steps

                    # ------- V first half (heads 0-7) + jo0 proj interleaved ----
                    with ExitStack() as p1b:
                        wvpool = p1b.enter_context(tc.tile_pool(name="w_v", bufs=1))
                        vpool0 = p1b.enter_context(tc.tile_pool(name="o_v", bufs=2))
                        pmmv = p1b.enter_context(
                            tc.tile_pool(name="ps_v", bufs=2, space="PSUM")
                        )
                        wv_t0 = wvpool.tile([P, DO, 512], F32R, tag="wv")
                        nc.gpsimd.dma_start(wv_t0[:], Wv_r[:, :, 0:512].bitcast(F32R))
                        ktp0, qtp0, steps0 = emit_proj_jo(0, preload=True)
                        for q in range(4):
                            eng = nc.sync if q % 2 == 0 else nc.gpsimd
                            eng.dma_start(
                                xts[:, :, q * 512 : (q + 1) * 512],
                                xT_r[:, :, q * 512 : (q + 1) * 512].bitcast(F32R),
                            )
                        nc.gpsimd.dma_start(ones128[:], ones128c.bitcast(F32R))
                        nc.gpsimd.dma_start(onesva[:], onescolc.bitcast(F32R))
                        nc.gpsimd.dma_start(permt[:], permc.bitcast(F32R))
                        nc.gpsimd.dma_start(ck[:], cosk[:])
                        nc.gpsimd.dma_start(sk[:], sink[:])
                        jo0_slots = {5: 0, 7: 1, 9: 2, 11: 3, 13: 4, 15: 5}
                        for to in range(NT):
                            ps = pmmv.tile([P, 512], F32, tag="psv")
                            for do in range(DO):
                                nc.tensor.matmul(
                                    ps[:],
                                    xts[:, do, to * P : (to + 1) * P],
                                    wv_t0[:, do],
                                    start=(do == 0),
                                    stop=(do == DO - 1),
                                )
                            vt = vpool0.tile([P, 512], F32R, tag="vt")
                            nc.vector.tensor_copy(vt[:], ps[:])
                            nc.sync.dma_start(
                                Vd3[0:8, :, to, 0:64]
                                .rearrange("h tp e -> tp h e")
                                .bitcast(F32R),
                                vt.rearrange("tp (h e) -> tp h e", e=64),
                            )
                            if to in jo0_slots:
                                steps0[jo0_slots[to]]()

                    vapool = pmain.enter_context(tc.tile_pool(name="va", bufs=2))
                    ptpool = pmain.enter_context(tc.tile_pool(name="pt", bufs=4))
                    smpool = pmain.enter_context(tc.tile_pool(name="sm", bufs=2))
                    pss = pmain.enter_context(
                        tc.tile_pool(name="ps_s", bufs=2, space="PSUM")
                    )
                    pso = pmain.enter_context(
                        tc.tile_pool(name="ps_o", bufs=2, space="PSUM")
                    )

                    wv1 = wv1pool.tile([P, DO, 512], F32R, tag="wv1")
                    nc.gpsimd.dma_start(wv1[:], Wv_r[:, :, 512:1024].bitcast(F32R))

                    def v_jc1_step(to):
                        ps = pmm.tile([P, 512], F32, tag="ps", name="psv1")
                        for do in range(DO):
                            nc.tensor.matmul(
                                ps[:],
                                xts[:, do, to * P : (to + 1) * P],
                                wv1[:, do],
                                start=(do == 0),
                                stop=(do == DO - 1),
                            )
                        vt = vpool.tile([P, 512], F32R, tag="vt1")
                        nc.vector.tensor_copy(vt[:], ps[:])
                        nc.sync.dma_start(
                            Vd3[8:16, :, to, 0:64]
                            .rearrange("h tp e -> tp h e")
                            .bitcast(F32R),
                            vt.rearrange("tp (h e) -> tp h e", e=64),
                        )

                    def emit_attention(p, ktp, qtp, interleave):
                        """Attention for head pair p. `interleave` is a list of
                        closures (next pair's projection steps) sprinkled between
                        blocks to keep the PE busy while ACT runs the exps."""
                        va = vapool.tile([P, NT, 65], F32R, tag="va")
                        nc.gpsimd.dma_start(
                            va[:, :, 0:64],
                            Vd3[2 * p, :, :, 0:64].bitcast(F32R),
                        )
                        nc.vector.tensor_copy(va[:, :, 64:65], onesva[:])
                        vb = vapool.tile([P, NT, 65], F32R, tag="vb")
                        nc.gpsimd.dma_start(
                            vb[:, :, 0:64],
                            Vd3[2 * p + 1, :, :, 0:64].bitcast(F32R),
                        )
                        nc.vector.tensor_copy(vb[:, :, 64:65], onesva[:])

                        il = list(interleave)
                        # sprinkle filler PE work (next pair's projections, V
                        # second-half) through the to-loop so the PE never drains
                        # while ACT runs the exps
                        n_it = 2 * NT
                        il_at = {}
                        for i in range(len(il)):
                            slot = 1 + (i * (n_it - 4)) // max(1, len(il))
                            while slot in il_at:
                                slot += 1
                            il_at[slot] = i
                        it_ctr = [0]

                        for lc in range(TQ // 512):
                            pos = [
                                pso.tile([P, 512], F32, tag=f"po{h}", name=f"po{h}")
                                for h in range(2)
                            ]
                            rows = [(0, 64), (64, P)]
                            vvs = [va, vb]
                            for to in range(NT):
                                pts = []
                                for h in range(2):
                                    r0, r1 = rows[h]
                                    ps = pss.tile([P, 512], F32, tag="pss")
                                    nc.tensor.matmul(
                                        ps[:],
                                        ktp[r0:r1, to * P : (to + 1) * P],
                                        qtp[r0:r1, lc * 512 : (lc + 1) * 512],
                                        start=True,
                                        stop=True,
                                    )
                                    pt = ptpool.tile([P, 512], F32R, tag="pt")
                                    nc.scalar.activation(
                                        pt[:], ps[:], AF.Exp, scale=SCALE
                                    )
                                    pts.append(pt)
                                for h in range(2):
                                    nc.tensor.matmul(
                                        pos[h][0:65, :],
                                        vvs[h][:, to, :],
                                        pts[h][:],
                                        start=(to == 0),
                                        stop=(to == NT - 1),
                                    )
                                k = it_ctr[0]
                                it_ctr[0] += 1
                                if k in il_at and il_at[k] < len(il):
                                    il[il_at[k]]()
                            for h in range(2):
                                po = pos[h]
                                # row 64 of po holds the softmax sums
                                rs = smpool.tile([P, 512], F32, tag="rs")
                                nc.vector.tensor_copy(rs[64:65, :], po[64:65, :])
                                rc = smpool.tile([P, 512], F32R, tag="rc")
                                with nc.allow_low_precision(
                                    reason="f32r feeds the broadcast matmul"
                                ):
                                    nc.vector.reciprocal(rc[64:65, :], rs[64:65, :])
                                pb = pmisc.tile([P, 512], F32, tag="misc", name="pb")[0:64, :]
                                nc.tensor.matmul(
                                    pb[:],
                                    ones128[64:65, :],
                                    rc[64:65, :],
                                    start=True,
                                    stop=True,
                                )
                                rb = smpool.tile([64, 512], F32, tag="rb")
                                nc.vector.tensor_copy(rb[:], pb[:])
                                tmpn = smpool.tile([64, 512], F32, tag="tmpn")
                                nc.vector.tensor_mul(tmpn[:], po[0:64, :], rb[:])
                                nc.sync.dma_start(
                                    AOTd[p, 64 * h : 64 * h + 64,
                                         lc * 512 : (lc + 1) * 512],
                                    tmpn[:],
                                )
                        done = {il_at[k] for k in il_at if il_at[k] < len(il)}
                        for i in range(len(il)):
                            if i not in done:
                                il[i]()

                    ktp, qtp = ktp0, qtp0
                    for p in range(DO):
                        if p + 1 < DO:
                            nktp, nqtp, nsteps = emit_proj_jo(p + 1)
                        else:
                            nktp = nqtp = None
                            nsteps = []
                        if p == 0:
                            nsteps = nsteps + [
                                (lambda to=to: v_jc1_step(to)) for to in range(6)
                            ]
                        elif p == 1:
                            nsteps = nsteps + [
                                (lambda to=to: v_jc1_step(to)) for to in range(6, 11)
                            ]
                        elif p == 2:
                            nsteps = nsteps + [
                                (lambda to=to: v_jc1_step(to)) for to in range(11, NT)
                            ]
                        emit_attention(p, ktp, qtp, nsteps)
                        ktp, qtp = nktp, nqtp

            # ---------------- output projection -------------------------------
            with ExitStack() as p3:
                wppool = p3.enter_context(tc.tile_pool(name="wp", bufs=1))
                wp_t = wppool.tile([P, DO, D], F32R)
                for ko in range(DO):
                    nc.sync.dma_start(wp_t[:, ko], Wp_r[:, ko].bitcast(F32R))
                bpt = wppool.tile([P, D], F32)
                nc.sync.dma_start(bpt[:], bpb[:])
                apool = p3.enter_context(tc.tile_pool(name="aot", bufs=3))
                outpool = p3.enter_context(tc.tile_pool(name="outp", bufs=3))
                ps_f = p3.enter_context(
                    tc.tile_pool(name="ps_f", bufs=4, space="PSUM")
                )
                for lt in range(TQ // P):
                    aot_t = apool.tile([P, DO, P], F32R, tag="aot")
                    nc.sync.dma_start(
                        aot_t[:], AOTd_r[:, :, lt * P : (lt + 1) * P].bitcast(F32R)
                    )
                    for jc in range(2):
                        ps = ps_f.tile([P, 512], F32, tag="psf")
                        for ko in range(DO):
                            nc.tensor.matmul(
                                ps[:],
                                aot_t[:, ko],
                                wp_t[:, ko, jc * 512 : (jc + 1) * 512],
                                start=(ko == 0),
                                stop=(ko == DO - 1),
                            )
                        ot = outpool.tile([P, 512], F32, tag="oto")
                        nc.vector.tensor_add(
                            ot[:], ps[:], bpt[:, jc * 512 : (jc + 1) * 512]
                        )
                        nc.sync.dma_start(
                            out[lt * P : (lt + 1) * P, jc * 512 : (jc + 1) * 512],
                            ot[:],
                        )

    if split_waits:
        _split_multi_waits(nc)
    return nc


def _rope_tables():
    inv = 1.0 / (ROPE_THETA ** (np.arange(0, HD, 2, dtype=np.float32) / HD))
    t = np.arange(T, dtype=np.float32)
    freqs = np.einsum("i,j->ij", t, inv)  # [T, 32]
    freqs = np.repeat(freqs, 2, axis=-1)  # [T, 64]
    cosT = np.cos(freqs).T  # [64, T]
    sinT = np.sin(freqs).T
    cosk = np.tile(cosT, (2, 1)).astype(np.float32)  # [128, T]
    sink = np.tile(sinT, (2, 1)).astype(np.float32)
    return np.ascontiguousarray(cosk), np.ascontiguousarray(sink)


_NC_CACHE = {}


def make_in_maps(x, Wq, Wk, Wv, Wp, bp):
    cosk, sink = _rope_tables()
    bpb = np.ascontiguousarray(np.tile(np.asarray(bp, np.float32)[None, :], (P, 1)))
    Wq = np.ascontiguousarray(np.asarray(Wq, np.float32))
    Wk = np.ascontiguousarray(np.asarray(Wk, np.float32))
    Wv = np.ascontiguousarray(np.asarray(Wv, np.float32))
    Wp = np.ascontiguousarray(np.asarray(Wp, np.float32))
    in_maps = []
    for c in range(8):
        b, qh = c // 2, c % 2
        xT = np.asarray(x[b], np.float32).T  # [D, T]
        roll = qh * TQ
        in_maps.append(
            {
                "xT": np.ascontiguousarray(np.roll(xT, -roll, axis=1)),
                "Wq": Wq,
                "Wk": Wk,
                "Wv": Wv,
                "Wp": Wp,
                "bpb": bpb,
                "cosk": np.ascontiguousarray(np.roll(cosk, -roll, axis=1)),
                "sink": np.ascontiguousarray(np.roll(sink, -roll, axis=1)),
            }
        )
    return in_maps


def kernel(x, h, w, Wq, Wk, Wv, Wp, bp, _trace=False, **trace_kwargs):
    x = np.asarray(x, np.float32)
    in_maps = make_in_maps(x, Wq, Wk, Wv, Wp, bp)
    if "nc" not in _NC_CACHE:
        _NC_CACHE["nc"] = build_nc()
    nc = _NC_CACHE["nc"]
    res = run_bass_kernel_spmd(
        nc, in_maps, list(range(8)), trace=_trace, **trace_kwargs
    )
    out = np.empty((B, T, D), np.float32)
    for c in range(8):
        b, qh = c // 2, c % 2
        out[b, qh * TQ : (qh + 1) * TQ, :] = res.results[c]["out"]
    kernel.last_result = res
    return out



# revision 5
# speedup vs baseline: 1.3240x; 1.3240x over previous
"""AttentionWithRoPE on 8 Trainium2 NeuronCores.

Sharding: batch x query-half -> 8 independent cores (no collectives).
Core c handles batch b=c//2, query rows [qh*1024, (qh+1)*1024) with qh=c%2.
The host rolls the t axis per core so the query rows always sit in columns
[0, TQ) of xT; cosk/sink are rolled identically, so RoPE sees true positions
and the softmax key set is unchanged (order-invariant).

Per-core plan:
  Projections in bf16 (x, Wq/Wk/Wv/Wp host-converted): PSUM f32 accum,
  RoPE via signed pair-swap permutation matmul + DVE combine, Q^T/K^T
  written as fp8e4 (quantization noise ~1% on the final output, well
  under the 2e-2 gate).
  S^T per head via ONE fp8 DoubleRow matmul (contraction 64 = 2 k-slots
  of the same 64 partitions: lhsT = K^T with stride-0 slot broadcast,
  rhs slot 1 points at a zero strip appended to qtp) -> 0.5 cycles/row.
  exp on ACT in [128, 1024] tiles (two heads' S share one 2-bank PSUM
  tile) -> bf16 P^T ring buffer.
  AV with l on partitions: out[l, d|1] = P^T-tile.T @ [V|1]-tile, 65-col
  moving operand (cost = 65 rows/matmul), accumulated over the 16 key
  tiles; row sums ride along as column 64.
  Softmax normalize via gpsimd normalize_recip (per-partition denom).
  attnout transposed to [j, l] via DMA-engine transpose (dma_start_transpose),
  then out = aotT.T @ Wp + bp in bf16.
  V kept SBUF-resident in bf16 (no DRAM staging); attnout likewise.
"""

import sys

sys.path.insert(0, "/opt/trn_rl_repo")

from contextlib import ExitStack

import numpy as np
import ml_dtypes

import concourse.bass as bass
import concourse.mybir as mybir
import concourse.tile as tile
from concourse.bass_utils import run_bass_kernel_spmd

F32 = mybir.dt.float32
F32R = mybir.dt.float32r
BF16 = mybir.dt.bfloat16
F8 = mybir.dt.float8e4
AF = mybir.ActivationFunctionType
MUL = mybir.AluOpType.mult
DR = mybir.MatmulPerfMode.DoubleRow

B, T, D = 4, 2048, 1024
H, HD = 16, 64
P = 128
TQ = 1024  # query rows per core
NT = T // P  # 16 key tiles
DO = D // P  # 8 contraction tiles
RING = 20  # P^T ring slots (>= NT + AV-lag slack)
SCALE = float(D) ** -0.5
ROPE_THETA = 10000.0

_ws_ctr = [0]


def _split_multi_waits(nc):
    """The walrus build in this container accepts at most one sync-wait per
    engine instruction. Hoist all but one wait of each instruction into
    standalone EventSemaphore instructions on the same engine, placed
    immediately before it (engines are in-order, so semantics are identical)."""
    n = 0
    for f in nc.m.functions:
        for blk in f.blocks:
            insts = list(blk.instructions)
            newlist = []
            changed = False
            for inst in insts:
                si = getattr(inst, "sync_info", None)
                waits = list(si.on_wait) if si is not None and si.on_wait else []
                if len(waits) > 1:
                    for w in waits[:-1]:
                        _ws_ctr[0] += 1
                        evs = mybir.InstEventSemaphore(
                            name=f"WSPLIT-{_ws_ctr[0]}", ins=[], outs=[]
                        )
                        evs.engine = inst.engine
                        evs.sync_info = mybir.SyncInfo(on_wait=[w], on_update=[])
                        newlist.append(evs)
                        n += 1
                    inst.sync_info = mybir.SyncInfo(
                        on_wait=[waits[-1]], on_update=list(si.on_update)
                    )
                    changed = True
                newlist.append(inst)
            if changed:
                blk.instructions[:] = newlist
    return n


def _perm_lhsT():
    """lhsT for the rotate-half permutation: out = lhsT.T @ q gives
    out[2i] = -q[2i+1], out[2i+1] = q[2i]."""
    m = np.zeros((P, P), np.float32)
    for i in range(P // 2):
        m[2 * i + 1, 2 * i] = -1.0
        m[2 * i, 2 * i + 1] = 1.0
    return m


def build_nc(split_waits=True, reps=1):
    nc = bass.Bass(trn_type="TRN2", target_bir_lowering=False, debug=False)

    xT = nc.dram_tensor("xT", [D, T], BF16, kind="ExternalInput").ap()
    Wq = nc.dram_tensor("Wq", [D, D], BF16, kind="ExternalInput").ap()
    Wk = nc.dram_tensor("Wk", [D, D], BF16, kind="ExternalInput").ap()
    Wv = nc.dram_tensor("Wv", [D, D], BF16, kind="ExternalInput").ap()
    Wp = nc.dram_tensor("Wp", [D, D], BF16, kind="ExternalInput").ap()
    bpb = nc.dram_tensor("bpb", [P, D], F32, kind="ExternalInput").ap()
    cosk = nc.dram_tensor("cosk", [P, T], BF16, kind="ExternalInput").ap()
    sink = nc.dram_tensor("sink", [P, T], BF16, kind="ExternalInput").ap()
    out = nc.dram_tensor("out", [TQ, D], F32, kind="ExternalOutput").ap()

    permc = nc.inline_tensor(_perm_lhsT(), name="permc").ap()

    xT_r = xT.rearrange("(do dp) t -> dp do t", dp=P)  # [128, 8, 2048]
    Wq_r = Wq.rearrange("(do dp) j -> dp do j", dp=P)
    Wk_r = Wk.rearrange("(do dp) j -> dp do j", dp=P)
    Wv_r = Wv.rearrange("(do dp) j -> dp do j", dp=P)
    Wp_r = Wp.rearrange("(ko kp) j -> kp ko j", kp=P)

    with tile.TileContext(nc) as tc:
      for _rep in range(reps):
        with ExitStack() as top:
            persist = top.enter_context(tc.tile_pool(name="persist", bufs=1))
            permf = persist.tile([P, P], F32R)
            permt = persist.tile([P, P], BF16)
            ck = persist.tile([P, T], BF16)
            sk = persist.tile([P, T], BF16)
            v8 = persist.tile([P, NT, H, 65], BF16)
            p8 = persist.tile([P, RING, 1024], BF16)
            aotT = persist.tile([P, DO, TQ], BF16)

            with tc.tile_pool(name="xpool", bufs=1) as xpool:
                xts = xpool.tile([P, DO, T], BF16)

                with ExitStack() as pmain:
                    wv1pool = pmain.enter_context(tc.tile_pool(name="w_v1", bufs=1))
                    vpool = pmain.enter_context(tc.tile_pool(name="o_v1", bufs=2))
                    wpool = pmain.enter_context(tc.tile_pool(name="w_qk", bufs=2))
                    tpool = pmain.enter_context(tc.tile_pool(name="t_qk", bufs=2))
                    kqpool = pmain.enter_context(tc.tile_pool(name="kq", bufs=2))
                    pmm = pmain.enter_context(
                        tc.tile_pool(name="ps_qk", bufs=1, space="PSUM")
                    )
                    pmisc = pmain.enter_context(
                        tc.tile_pool(name="ps_misc", bufs=1, space="PSUM")
                    )

                    def proj_rope(w_t, dst, dst_col, src_col):
                        """One [128, 512] Q/K projection tile + RoPE into dst
                        (fp8)."""
                        ps = pmm.tile([P, 512], F32, tag="ps")
                        for do in range(DO):
                            nc.tensor.matmul(
                                ps[:],
                                w_t[:, do],
                                xts[:, do, src_col * 512 : (src_col + 1) * 512],
                                start=(do == 0),
                                stop=(do == DO - 1),
                            )
                        raw = tpool.tile([P, 512], BF16, tag="raw")
                        nc.vector.tensor_copy(raw[:], ps[:])
                        pr = pmisc.tile([P, 512], F32, tag="misc")
                        nc.tensor.matmul(pr[:], permt[:], raw[:], start=True, stop=True)
                        t1 = tpool.tile([P, 512], BF16, tag="t1")
                        nc.vector.tensor_mul(
                            t1[:], raw[:], ck[:, src_col * 512 : (src_col + 1) * 512]
                        )
                        t2 = tpool.tile([P, 512], BF16, tag="t2")
                        nc.vector.tensor_mul(
                            t2[:], pr[:], sk[:, src_col * 512 : (src_col + 1) * 512]
                        )
                        nc.vector.tensor_add(
                            dst[:, dst_col * 512 : (dst_col + 1) * 512], t1[:], t2[:]
                        )

                    def emit_proj_jo(jo, preload=False):
                        """Q^T/K^T (fp8) for head pair jo; returns closures."""
                        ktp = kqpool.tile([P, T], F8, tag="ktp")
                        qtp = kqpool.tile([P, TQ + 512], F8, tag="qtp")
                        steps = []

                        def load_w():
                            wq_t = wpool.tile([P, DO, P], BF16, tag="wq")
                            nc.gpsimd.dma_start(
                                wq_t[:], Wq_r[:, :, jo * P : (jo + 1) * P]
                            )
                            wk_t = wpool.tile([P, DO, P], BF16, tag="wk")
                            nc.gpsimd.dma_start(
                                wk_t[:], Wk_r[:, :, jo * P : (jo + 1) * P]
                            )
                            nc.vector.memset(qtp[:, TQ : TQ + 512], 0.0)
                            return wq_t, wk_t

                        wref = []
                        if preload:
                            wref.extend(load_w())

                        def step(i):
                            if i == 0 and not wref:
                                wref.extend(load_w())
                            wq_t, wk_t = wref
                            if i < 2:
                                proj_rope(wq_t, qtp, i, i)
                            else:
                                proj_rope(wk_t, ktp, i - 2, i - 2)

                        for i in range(6):
                            steps.append(lambda i=i: step(i))
                        return ktp, qtp, steps

                    # ------- V first half (heads 0-7) + jo0 proj interleaved --
                    with ExitStack() as p1b:
                        wvpool = p1b.enter_context(tc.tile_pool(name="w_v", bufs=1))
                        pmmv = p1b.enter_context(
                            tc.tile_pool(name="ps_v", bufs=2, space="PSUM")
                        )
                        wv_t0 = wvpool.tile([P, DO, 512], BF16, tag="wv")
                        nc.gpsimd.dma_start(wv_t0[:], Wv_r[:, :, 0:512])
                        ktp0, qtp0, steps0 = emit_proj_jo(0, preload=True)
                        for q in range(4):
                            eng = nc.sync if q % 2 == 0 else nc.gpsimd
                            eng.dma_start(
                                xts[:, :, q * 512 : (q + 1) * 512],
                                xT_r[:, :, q * 512 : (q + 1) * 512],
                            )
                        nc.gpsimd.dma_start(permf[:], permc.bitcast(F32R))
                        nc.vector.tensor_copy(permt[:], permf.bitcast(F32))
                        nc.gpsimd.dma_start(ck[:], cosk[:])
                        nc.gpsimd.dma_start(sk[:], sink[:])
                        nc.vector.memset(v8[:, :, :, 64:65], 1.0)
                        jo0_slots = {5: 0, 7: 1, 9: 2, 11: 3, 13: 4, 15: 5}
                        for to in range(NT):
                            ps = pmmv.tile([P, 512], F32, tag="psv")
                            for do in range(DO):
                                nc.tensor.matmul(
                                    ps[:],
                                    xts[:, do, to * P : (to + 1) * P],
                                    wv_t0[:, do],
                                    start=(do == 0),
                                    stop=(do == DO - 1),
                                )
                            nc.vector.tensor_copy(
                                v8[:, to, 0:8, 0:64],
                                ps.rearrange("tp (h e) -> tp h e", e=64),
                            )
                            if to in jo0_slots:
                                steps0[jo0_slots[to]]()

                    apool = pmain.enter_context(tc.tile_pool(name="aotp", bufs=2))
                    smpool = pmain.enter_context(tc.tile_pool(name="sm", bufs=3))
                    pss = pmain.enter_context(
                        tc.tile_pool(name="ps_s", bufs=2, space="PSUM")
                    )
                    pav = pmain.enter_context(
                        tc.tile_pool(name="ps_av", bufs=2, space="PSUM")
                    )

                    wv1 = wv1pool.tile([P, DO, 512], BF16, tag="wv1")
                    nc.gpsimd.dma_start(wv1[:], Wv_r[:, :, 512:1024])

                    def v_jc1_step(to):
                        ps = pmm.tile([P, 512], F32, tag="ps", name="psv1")
                        for do in range(DO):
                            nc.tensor.matmul(
                                ps[:],
                                xts[:, do, to * P : (to + 1) * P],
                                wv1[:, do],
                                start=(do == 0),
                                stop=(do == DO - 1),
                            )
                        nc.vector.tensor_copy(
                            v8[:, to, 8:16, 0:64],
                            ps.rearrange("tp (h e) -> tp h e", e=64),
                        )

                    def emit_attention(p, ktp, qtp, interleave):
                        """Attention for head pair p. `interleave` closures are
                        sprinkled between to-units to keep the PE busy while
                        ACT grinds the exps."""
                        q3 = qtp.rearrange("d (s l) -> d s l", s=3)
                        aotP = apool.tile([P, TQ // P, P], BF16, tag="aotP")

                        il = list(interleave)
                        n_it = 2 * NT
                        il_at = {}
                        for i in range(len(il)):
                            slot = 1 + (i * (n_it - 4)) // max(1, len(il))
                            while slot in il_at:
                                slot += 1
                            il_at[slot] = i
                        it_ctr = [0]

                        for lc in range(2):
                            for to in range(NT):
                                s = (p * 32 + lc * NT + to) % RING
                                pst = pss.tile([P, 1024], F32, tag="pss")
                                for h in range(2):
                                    r0 = 64 * h
                                    lhsT = (
                                        ktp[r0 : r0 + 64, to * P : (to + 1) * P]
                                        .unsqueeze(1)
                                        .broadcast_to([64, 2, P])
                                    )
                                    if lc == 0:
                                        rhs = q3[r0 : r0 + 64, 0::2, :]
                                    else:
                                        rhs = q3[r0 : r0 + 64, 1:3, :]
                                    nc.tensor.matmul(
                                        pst[:, 512 * h : 512 * h + 512],
                                        lhsT,
                                        rhs,
                                        start=True,
                                        stop=True,
                                        perf_mode=DR,
                                    )
                                nc.scalar.activation(
                                    p8[:, s, :], pst[:], AF.Exp, scale=SCALE
                                )
                                k = it_ctr[0]
                                it_ctr[0] += 1
                                if k in il_at and il_at[k] < len(il):
                                    il[il_at[k]]()
                            # AV + normalize for this lc
                            for h in range(2):
                                for lt in range(4):
                                    ltg = lc * 4 + lt
                                    pav_t = pav.tile([P, 512], F32, tag="pav")
                                    for to in range(NT):
                                        s = (p * 32 + lc * NT + to) % RING
                                        nc.tensor.matmul(
                                            pav_t[:, 0:65],
                                            p8[
                                                :,
                                                s,
                                                512 * h + lt * P : 512 * h
                                                + (lt + 1) * P,
                                            ],
                                            v8[:, to, 2 * p + h, :],
                                            start=(to == 0),
                                            stop=(to == NT - 1),
                                        )
                                    avs = smpool.tile([P, 65], F32, tag="avs")
                                    nc.vector.tensor_copy(avs[:], pav_t[:, 0:65])
                                    rc = smpool.tile([P, 1], F32, tag="rc")
                                    with nc.allow_low_precision(
                                        reason="softmax denom reciprocal"
                                    ):
                                        nc.vector.reciprocal(rc[:], avs[:, 64:65])
                                    nc.vector.tensor_scalar_mul(
                                        aotP[:, ltg, 64 * h : 64 * h + 64],
                                        avs[:, 0:64],
                                        rc[:],
                                    )
                        # transposes to aotT
                        for ltg in range(TQ // P):
                            nc.sync.dma_start_transpose(
                                aotT[:, p, ltg * P : (ltg + 1) * P],
                                aotP[:, ltg, :],
                            )
                        done = {il_at[k] for k in il_at if il_at[k] < len(il)}
                        for i in range(len(il)):
                            if i not in done:
                                il[i]()

                    ktp, qtp = ktp0, qtp0
                    for p in range(DO):
                        if p + 1 < DO:
                            nktp, nqtp, nsteps = emit_proj_jo(p + 1)
                        else:
                            nktp = nqtp = None
                            nsteps = []
                        if p == 0:
                            nsteps = nsteps + [
                                (lambda to=to: v_jc1_step(to)) for to in range(6)
                            ]
                        elif p == 1:
                            nsteps = nsteps + [
                                (lambda to=to: v_jc1_step(to)) for to in range(6, 11)
                            ]
                        elif p == 2:
                            nsteps = nsteps + [
                                (lambda to=to: v_jc1_step(to)) for to in range(11, NT)
                            ]
                        emit_attention(p, ktp, qtp, nsteps)
                        ktp, qtp = nktp, nqtp

            # ---------------- output projection -------------------------------
            with ExitStack() as p3:
                wppool = p3.enter_context(tc.tile_pool(name="wp", bufs=1))
                wp_t = wppool.tile([P, DO, D], BF16)
                for ko in range(DO):
                    nc.sync.dma_start(wp_t[:, ko], Wp_r[:, ko])
                bpt = wppool.tile([P, D], F32)
                nc.sync.dma_start(bpt[:], bpb[:])
                outpool = p3.enter_context(tc.tile_pool(name="outp", bufs=3))
                ps_f = p3.enter_context(
                    tc.tile_pool(name="ps_f", bufs=4, space="PSUM")
                )
                for lt in range(TQ // P):
                    for jc in range(2):
                        ps = ps_f.tile([P, 512], F32, tag="psf")
                        for ko in range(DO):
                            nc.tensor.matmul(
                                ps[:],
                                aotT[:, ko, lt * P : (lt + 1) * P],
                                wp_t[:, ko, jc * 512 : (jc + 1) * 512],
                                start=(ko == 0),
                                stop=(ko == DO - 1),
                            )
                        ot = outpool.tile([P, 512], F32, tag="oto")
                        nc.vector.tensor_add(
                            ot[:], ps[:], bpt[:, jc * 512 : (jc + 1) * 512]
                        )
                        nc.sync.dma_start(
                            out[lt * P : (lt + 1) * P, jc * 512 : (jc + 1) * 512],
                            ot[:],
                        )

    if split_waits:
        _split_multi_waits(nc)
    return nc


def _rope_tables():
    inv = 1.0 / (ROPE_THETA ** (np.arange(0, HD, 2, dtype=np.float32) / HD))
    t = np.arange(T, dtype=np.float32)
    freqs = np.einsum("i,j->ij", t, inv)  # [T, 32]
    freqs = np.repeat(freqs, 2, axis=-1)  # [T, 64]
    cosT = np.cos(freqs).T  # [64, T]
    sinT = np.sin(freqs).T
    cosk = np.tile(cosT, (2, 1)).astype(np.float32)  # [128, T]
    sink = np.tile(sinT, (2, 1)).astype(np.float32)
    return np.ascontiguousarray(cosk), np.ascontiguousarray(sink)


_NC_CACHE = {}


def make_in_maps(x, Wq, Wk, Wv, Wp, bp):
    cosk, sink = _rope_tables()
    bpb = np.ascontiguousarray(np.tile(np.asarray(bp, np.float32)[None, :], (P, 1)))
    bf = lambda a: np.ascontiguousarray(np.asarray(a, np.float32)).astype(
        ml_dtypes.bfloat16
    )
    Wqb, Wkb, Wvb, Wpb = bf(Wq), bf(Wk), bf(Wv), bf(Wp)
    in_maps = []
    for c in range(8):
        b, qh = c // 2, c % 2
        xT = np.asarray(x[b], np.float32).T  # [D, T]
        roll = qh * TQ
        in_maps.append(
            {
                "xT": bf(np.roll(xT, -roll, axis=1)),
                "Wq": Wqb,
                "Wk": Wkb,
                "Wv": Wvb,
                "Wp": Wpb,
                "bpb": bpb,
                "cosk": bf(np.roll(cosk, -roll, axis=1)),
                "sink": bf(np.roll(sink, -roll, axis=1)),
            }
        )
    return in_maps


def kernel(x, h, w, Wq, Wk, Wv, Wp, bp, _trace=False, **trace_kwargs):
    x = np.asarray(x, np.float32)
    in_maps = make_in_maps(x, Wq, Wk, Wv, Wp, bp)
    if "nc" not in _NC_CACHE:
        _NC_CACHE["nc"] = build_nc()
    nc = _NC_CACHE["nc"]
    res = run_bass_kernel_spmd(
        nc, in_maps, list(range(8)), trace=_trace, **trace_kwargs
    )
    out = np.empty((B, T, D), np.float32)
    for c in range(8):
        b, qh = c // 2, c % 2
        out[b, qh * TQ : (qh + 1) * TQ, :] = res.results[c]["out"]
    kernel.last_result = res
    return out


# revision 10
# speedup vs baseline: 1.4833x; 1.1204x over previous
"""AttentionWithRoPE on 8 Trainium2 NeuronCores.

Sharding: batch x query-half -> 8 independent cores (no collectives).
Core c handles batch b=c//2, query rows [qh*1024, (qh+1)*1024) with qh=c%2.
The host rolls the t axis per core so the query rows always sit in columns
[0, TQ) of xT; cosk/sink are rolled identically, so RoPE sees true positions
and the softmax key set is unchanged (order-invariant).

Per-core plan:
  Projections in bf16 (x, Wq/Wk/Wv/Wp host-converted): PSUM f32 accum,
  RoPE via signed pair-swap permutation matmul + DVE combine, Q^T/K^T
  written as fp8e4 (quantization noise ~1% on the final output, well
  under the 2e-2 gate).
  S^T per head via ONE fp8 DoubleRow matmul (contraction 64 = 2 k-slots
  of the same 64 partitions: lhsT = K^T with stride-0 slot broadcast,
  rhs slot 1 points at a zero strip appended to qtp) -> 0.5 cycles/row.
  exp on ACT in [128, 1024] tiles (two heads' S share one 2-bank PSUM
  tile) -> bf16 P^T ring buffer.
  AV with l on partitions: out[l, d|1] = P^T-tile.T @ [V|1]-tile, 65-col
  moving operand (cost = 65 rows/matmul), accumulated over the 16 key
  tiles; row sums ride along as column 64.
  Softmax normalize via gpsimd normalize_recip (per-partition denom).
  attnout transposed to [j, l] via DMA-engine transpose (dma_start_transpose),
  then out = aotT.T @ Wp + bp in bf16.
  V kept SBUF-resident in bf16 (no DRAM staging); attnout likewise.
"""

import sys

sys.path.insert(0, "/opt/trn_rl_repo")

from contextlib import ExitStack

import numpy as np
import ml_dtypes

import concourse.bass as bass
import concourse.mybir as mybir
import concourse.tile as tile
from concourse.bass_utils import run_bass_kernel_spmd

F32 = mybir.dt.float32
F32R = mybir.dt.float32r
BF16 = mybir.dt.bfloat16
F8 = mybir.dt.float8e4
AF = mybir.ActivationFunctionType
MUL = mybir.AluOpType.mult
DR = mybir.MatmulPerfMode.DoubleRow

B, T, D = 4, 2048, 1024
H, HD = 16, 64
P = 128
TQ = 1024  # query rows per core
NT = T // P  # 16 key tiles
DO = D // P  # 8 contraction tiles
RING = 26  # P^T ring slots (16 live + AV-closure lag, see emit_attention)
SCALE = float(D) ** -0.5
ROPE_THETA = 10000.0

_ws_ctr = [0]


def _split_multi_waits(nc):
    """The walrus build in this container accepts at most one sync-wait per
    engine instruction. Hoist all but one wait of each instruction into
    standalone EventSemaphore instructions on the same engine, placed
    immediately before it (engines are in-order, so semantics are identical)."""
    n = 0
    for f in nc.m.functions:
        for blk in f.blocks:
            insts = list(blk.instructions)
            newlist = []
            changed = False
            for inst in insts:
                si = getattr(inst, "sync_info", None)
                waits = list(si.on_wait) if si is not None and si.on_wait else []
                if len(waits) > 1:
                    for w in waits[:-1]:
                        _ws_ctr[0] += 1
                        evs = mybir.InstEventSemaphore(
                            name=f"WSPLIT-{_ws_ctr[0]}", ins=[], outs=[]
                        )
                        evs.engine = inst.engine
                        evs.sync_info = mybir.SyncInfo(on_wait=[w], on_update=[])
                        newlist.append(evs)
                        n += 1
                    inst.sync_info = mybir.SyncInfo(
                        on_wait=[waits[-1]], on_update=list(si.on_update)
                    )
                    changed = True
                newlist.append(inst)
            if changed:
                blk.instructions[:] = newlist
    return n


def _perm_lhsT():
    """lhsT for the rotate-half permutation: out = lhsT.T @ q gives
    out[2i] = -q[2i+1], out[2i+1] = q[2i]."""
    m = np.zeros((P, P), np.float32)
    for i in range(P // 2):
        m[2 * i + 1, 2 * i] = -1.0
        m[2 * i, 2 * i + 1] = 1.0
    return m


def build_nc(split_waits=True, reps=1):
    nc = bass.Bass(trn_type="TRN2", target_bir_lowering=False, debug=False)

    xT = nc.dram_tensor("xT", [D, T], BF16, kind="ExternalInput").ap()
    Wq = nc.dram_tensor("Wq", [D, D], BF16, kind="ExternalInput").ap()
    Wk = nc.dram_tensor("Wk", [D, D], BF16, kind="ExternalInput").ap()
    Wv = nc.dram_tensor("Wv", [D, D], BF16, kind="ExternalInput").ap()
    Wp = nc.dram_tensor("Wp", [D, D], BF16, kind="ExternalInput").ap()
    bpb = nc.dram_tensor("bpb", [P, D], F32, kind="ExternalInput").ap()
    cosk = nc.dram_tensor("cosk", [P, T], BF16, kind="ExternalInput").ap()
    sink = nc.dram_tensor("sink", [P, T], BF16, kind="ExternalInput").ap()
    out = nc.dram_tensor("out", [TQ, D], F32, kind="ExternalOutput").ap()

    permc = nc.inline_tensor(_perm_lhsT(), name="permc").ap()

    xT_r = xT.rearrange("(do dp) t -> dp do t", dp=P)  # [128, 8, 2048]
    Wq_r = Wq.rearrange("(do dp) j -> dp do j", dp=P)
    Wk_r = Wk.rearrange("(do dp) j -> dp do j", dp=P)
    Wv_r = Wv.rearrange("(do dp) j -> dp do j", dp=P)
    Wp_r = Wp.rearrange("(ko kp) j -> kp ko j", kp=P)

    with tile.TileContext(nc) as tc:
      for _rep in range(reps):
        with ExitStack() as top:
            persist = top.enter_context(tc.tile_pool(name="persist", bufs=1))
            permf = persist.tile([P, P], F32R)
            permt = persist.tile([P, P], BF16)
            ck = persist.tile([P, T], BF16)
            sk = persist.tile([P, T], BF16)
            v8 = persist.tile([P, NT, H, 65], BF16)
            p8 = persist.tile([P, RING, 1024], BF16)
            aotT = persist.tile([P, DO, TQ], BF16)

            with tc.tile_pool(name="xpool", bufs=1) as xpool:
                xts = xpool.tile([P, DO, T], BF16)

                with ExitStack() as pmain:
                    wvgpool = pmain.enter_context(tc.tile_pool(name="w_vg", bufs=2))
                    wpool = pmain.enter_context(tc.tile_pool(name="w_qk", bufs=2))
                    tpool = pmain.enter_context(tc.tile_pool(name="t_qk", bufs=2))
                    kqpool = pmain.enter_context(tc.tile_pool(name="kq", bufs=2))
                    pmm = pmain.enter_context(
                        tc.tile_pool(name="ps_qk", bufs=1, space="PSUM")
                    )
                    pmisc = pmain.enter_context(
                        tc.tile_pool(name="ps_misc", bufs=1, space="PSUM")
                    )

                    def proj_rope_halves(w_t, dst, dst_col, src_col):
                        """One [128, 512] Q/K projection tile + RoPE into dst
                        (fp8), split into two closures so the PE burst
                        interleaves finely with the exp street."""
                        ref = {}

                        def ha():
                            ps = pmm.tile([P, 512], F32, tag="ps")
                            ref["ps"] = ps
                            for do in range(4):
                                nc.tensor.matmul(
                                    ps[:],
                                    w_t[:, do],
                                    xts[:, do, src_col * 512 : (src_col + 1) * 512],
                                    start=(do == 0),
                                    stop=False,
                                )

                        def hb():
                            ps = ref["ps"]
                            for do in range(4, DO):
                                nc.tensor.matmul(
                                    ps[:],
                                    w_t[:, do],
                                    xts[:, do, src_col * 512 : (src_col + 1) * 512],
                                    start=False,
                                    stop=(do == DO - 1),
                                )
                            raw = tpool.tile([P, 512], BF16, tag="raw")
                            nc.vector.tensor_copy(raw[:], ps[:])
                            pr = pmisc.tile([P, 512], F32, tag="misc")
                            nc.tensor.matmul(
                                pr[:], permt[:], raw[:], start=True, stop=True
                            )
                            t1 = tpool.tile([P, 512], BF16, tag="t1")
                            nc.vector.tensor_mul(
                                t1[:],
                                raw[:],
                                ck[:, src_col * 512 : (src_col + 1) * 512],
                            )
                            t2 = tpool.tile([P, 512], BF16, tag="t2")
                            nc.vector.tensor_mul(
                                t2[:], pr[:], sk[:, src_col * 512 : (src_col + 1) * 512]
                            )
                            nc.vector.tensor_add(
                                dst[:, dst_col * 512 : (dst_col + 1) * 512],
                                t1[:],
                                t2[:],
                            )

                        return [ha, hb]

                    def emit_proj_jo(jo, preload=False):
                        """Q^T/K^T (fp8) for head pair jo; returns 12 half-step
                        closures: q0a q0b q1a q1b k0a..k3b."""
                        ktp = kqpool.tile([P, T], F8, tag="ktp")
                        qtp = kqpool.tile([P, TQ + 512], F8, tag="qtp")

                        def load_w():
                            wq_t = wpool.tile([P, DO, P], BF16, tag="wq")
                            nc.gpsimd.dma_start(
                                wq_t[:], Wq_r[:, :, jo * P : (jo + 1) * P]
                            )
                            wk_t = wpool.tile([P, DO, P], BF16, tag="wk")
                            nc.gpsimd.dma_start(
                                wk_t[:], Wk_r[:, :, jo * P : (jo + 1) * P]
                            )
                            nc.vector.memset(qtp[:, TQ : TQ + 512], 0.0)
                            return wq_t, wk_t

                        wref = []
                        if preload:
                            wref.extend(load_w())

                        halves = []
                        for i in range(6):
                            ref = {}

                            def mk(i=i, ref=ref):
                                def lazy_a():
                                    if not wref:
                                        wref.extend(load_w())
                                    wq_t, wk_t = wref
                                    if i < 2:
                                        hs = proj_rope_halves(wq_t, qtp, i, i)
                                    else:
                                        hs = proj_rope_halves(
                                            wk_t, ktp, i - 2, i - 2
                                        )
                                    ref["hs"] = hs
                                    hs[0]()

                                def lazy_b():
                                    ref["hs"][1]()

                                return [lazy_a, lazy_b]

                            halves.extend(mk())
                        return ktp, qtp, halves

                    def load_wv(g):
                        wv_t = wvgpool.tile([P, DO, 256], BF16, tag="wvg")
                        nc.gpsimd.dma_start(
                            wv_t[:], Wv_r[:, :, g * 256 : (g + 1) * 256]
                        )
                        return wv_t

                    def v_step(wv_t, g, to):
                        """V projection for head group g (4 heads), key tile to."""
                        ps = pmisc.tile([P, 512], F32, tag="misc", name="psv")
                        for do in range(DO):
                            nc.tensor.matmul(
                                ps[:, 0:256],
                                xts[:, do, to * P : (to + 1) * P],
                                wv_t[:, do],
                                start=(do == 0),
                                stop=(do == DO - 1),
                            )
                        nc.vector.tensor_copy(
                            v8[:, to, 4 * g : 4 * g + 4, 0:64],
                            ps[:, 0:256].rearrange("tp (h e) -> tp h e", e=64),
                        )

                    # ------- phase A: V group 0 (to 0..7) + pair-0 Q/K proj ----
                    with ExitStack() as p1b:
                        pmmv = p1b.enter_context(
                            tc.tile_pool(name="ps_v", bufs=2, space="PSUM")
                        )
                        nc.sync.dma_start(xts[:, :, 0:512], xT_r[:, :, 0:512])
                        wv_g0 = load_wv(0)
                        nc.gpsimd.dma_start(
                            xts[:, :, 512:1024], xT_r[:, :, 512:1024]
                        )
                        ktp0, qtp0, halves0 = emit_proj_jo(0, preload=True)
                        nc.sync.dma_start(ck[:], cosk[:])
                        nc.sync.dma_start(sk[:], sink[:])
                        nc.gpsimd.dma_start(
                            xts[:, :, 1536:2048], xT_r[:, :, 1536:2048]
                        )
                        nc.sync.dma_start(
                            xts[:, :, 1024:1536], xT_r[:, :, 1024:1536]
                        )
                        nc.gpsimd.dma_start(permf[:], permc.bitcast(F32R))
                        nc.vector.tensor_copy(permt[:], permf.bitcast(F32))
                        nc.vector.memset(v8[:, :, :, 64:65], 1.0)

                        def v0_step(to):
                            ps = pmmv.tile([P, 512], F32, tag="psv")
                            for do in range(DO):
                                nc.tensor.matmul(
                                    ps[:, 0:256],
                                    xts[:, do, to * P : (to + 1) * P],
                                    wv_g0[:, do],
                                    start=(do == 0),
                                    stop=(do == DO - 1),
                                )
                            nc.vector.tensor_copy(
                                v8[:, to, 0:4, 0:64],
                                ps[:, 0:256].rearrange("tp (h e) -> tp h e", e=64),
                            )

                        # proj-0: q0 (halves 0,1) + k0..k3 (halves 4..11);
                        # q1 (halves 2,3) is deferred into attention(0).
                        pa_proj = [halves0[i] for i in (0, 1, 4, 5, 6, 7, 8, 9, 10, 11)]
                        # V tiles 0-3 run while the wq/wk loads land; the proj
                        # halves then interleave with the rest.
                        seq = [0, 1, 2, 3, "p0", "p1", "p2", "p3", 4, 5,
                               "p4", "p5", 6, 7, "p6", "p7", "p8", "p9"]
                        for item in seq:
                            if isinstance(item, int):
                                v0_step(item)
                            else:
                                pa_proj[int(item[1:])]()

                    apool = pmain.enter_context(tc.tile_pool(name="aotp", bufs=2))
                    smpool = pmain.enter_context(tc.tile_pool(name="sm", bufs=3))
                    pss = pmain.enter_context(
                        tc.tile_pool(name="ps_s", bufs=2, space="PSUM")
                    )
                    pav = pmain.enter_context(
                        tc.tile_pool(name="ps_av", bufs=2, space="PSUM")
                    )

                    def emit_attention(p, ktp, qtp, av_prev, tr_prev, work):
                        """Attention for head pair p.

                        av_prev: 16 AV-stream closures of pair p-1's second
                        query chunk -- run at this pair's lc0 units 0..7.
                        tr_prev: pair p-1's transpose closure (unit 8).
                        work: proj/V closures for upcoming pairs, spread over
                        units 8..15 of both chunks.
                        Returns (av_lc1_closures, tr_closure) for pair p.
                        """
                        q3 = qtp.rearrange("d (s l) -> d s l", s=3)
                        aotP = apool.tile([P, TQ // P, P], BF16, tag="aotP")

                        def av_stream(lc, h, lt):
                            ltg = lc * 4 + lt
                            pav_t = pav.tile([P, 512], F32, tag="pav")
                            for to in range(NT):
                                s = (p * 32 + lc * NT + to) % RING
                                nc.tensor.matmul(
                                    pav_t[:, 0:65],
                                    p8[
                                        :,
                                        s,
                                        512 * h + lt * P : 512 * h + (lt + 1) * P,
                                    ],
                                    v8[:, to, 2 * p + h, :],
                                    start=(to == 0),
                                    stop=(to == NT - 1),
                                )
                            avs = smpool.tile([P, 65], F32, tag="avs")
                            nc.vector.tensor_copy(avs[:], pav_t[:, 0:65])
                            rc = smpool.tile([P, 1], F32, tag="rc")
                            with nc.allow_low_precision(
                                reason="softmax denom reciprocal"
                            ):
                                nc.vector.reciprocal(rc[:], avs[:, 64:65])
                            nc.vector.tensor_scalar_mul(
                                aotP[:, ltg, 64 * h : 64 * h + 64],
                                avs[:, 0:64],
                                rc[:],
                            )

                        av_cl = {
                            lc: [
                                (lambda lc=lc, h=h, lt=lt: av_stream(lc, h, lt))
                                for h in range(2)
                                for lt in range(4)
                            ]
                            for lc in range(2)
                        }

                        def tr_closure():
                            for ltg in range(TQ // P):
                                nc.sync.dma_start_transpose(
                                    aotT[:, p, ltg * P : (ltg + 1) * P],
                                    aotP[:, ltg, :],
                                )

                        # slot schedule: unit -> closures
                        sched = {u: [] for u in range(32)}
                        if av_prev is not None:
                            for i, cl in enumerate(av_prev):
                                sched[i // 2].append(cl)
                        for i, cl in enumerate(av_cl[0]):
                            sched[16 + i // 2].append(cl)
                        if tr_prev is not None:
                            sched[8].append(tr_prev)
                        wslots = list(range(8, 16)) + list(range(24, 32))
                        if av_prev is None:
                            wslots = list(range(0, 16)) + list(range(24, 32))
                        wq = list(work)
                        for u in wslots:
                            while wq and len(sched[u]) < 2:
                                sched[u].append(wq.pop(0))
                        assert not wq, f"work overflow pair {p}: {len(wq)} left"

                        for lc in range(2):
                            for to in range(NT):
                                s = (p * 32 + lc * NT + to) % RING
                                pst = pss.tile([P, 1024], F32, tag="pss")
                                for h in range(2):
                                    r0 = 64 * h
                                    lhsT = (
                                        ktp[r0 : r0 + 64, to * P : (to + 1) * P]
                                        .unsqueeze(1)
                                        .broadcast_to([64, 2, P])
                                    )
                                    if lc == 0:
                                        rhs = q3[r0 : r0 + 64, 0::2, :]
                                    else:
                                        rhs = q3[r0 : r0 + 64, 1:3, :]
                                    nc.tensor.matmul(
                                        pst[:, 512 * h : 512 * h + 512],
                                        lhsT,
                                        rhs,
                                        start=True,
                                        stop=True,
                                        perf_mode=DR,
                                    )
                                nc.scalar.activation(
                                    p8[:, s, :], pst[:], AF.Exp, scale=SCALE
                                )
                                for cl in sched[lc * NT + to]:
                                    cl()
                        return av_cl[1], tr_closure

                    # V group g (heads 4g..4g+3) is produced across attentions
                    # 2g-2 and 2g-1; its consumers are pairs 2g, 2g+1.
                    wv_ref = {}
                    ktp, qtp = ktp0, qtp0
                    av_prev = None
                    tr_prev = None
                    for p in range(DO):
                        work = []
                        if p == 0:
                            work += [halves0[2], halves0[3]]  # q1 of pair 0
                            work += [
                                (lambda to=to: v_step(wv_g0, 0, to))
                                for to in range(8, NT)
                            ]
                        if p >= 1:
                            g = (p + 1) // 2
                            if g <= 3:
                                if p % 2 == 1:
                                    wv_ref[g] = load_wv(g)
                                    rng_to = range(0, 8)
                                else:
                                    rng_to = range(8, NT)
                                wv_t = wv_ref[g]
                                work += [
                                    (lambda to=to, wv_t=wv_t, g=g: v_step(
                                        wv_t, g, to
                                    ))
                                    for to in rng_to
                                ]
                        if p + 1 < DO:
                            nktp, nqtp, nhalves = emit_proj_jo(p + 1)
                            work += nhalves
                        else:
                            nktp = nqtp = None
                        av_prev, tr_prev = emit_attention(
                            p, ktp, qtp, av_prev, tr_prev, work
                        )
                        ktp, qtp = nktp, nqtp
                    # tail: pair 7 second-chunk AV + its transposes
                    for cl in av_prev:
                        cl()
                    tr_prev()

            # ---------------- output projection -------------------------------
            with ExitStack() as p3:
                wppool = p3.enter_context(tc.tile_pool(name="wp", bufs=1))
                wp_t = wppool.tile([P, DO, D], BF16)
                for ko in range(DO):
                    nc.sync.dma_start(wp_t[:, ko], Wp_r[:, ko])
                bpt = wppool.tile([P, D], F32)
                nc.sync.dma_start(bpt[:], bpb[:])
                outpool = p3.enter_context(tc.tile_pool(name="outp", bufs=3))
                ps_f = p3.enter_context(
                    tc.tile_pool(name="ps_f", bufs=4, space="PSUM")
                )
                for lt in range(TQ // P):
                    for jc in range(2):
                        ps = ps_f.tile([P, 512], F32, tag="psf")
                        for ko in range(DO):
                            nc.tensor.matmul(
                                ps[:],
                                aotT[:, ko, lt * P : (lt + 1) * P],
                                wp_t[:, ko, jc * 512 : (jc + 1) * 512],
                                start=(ko == 0),
                                stop=(ko == DO - 1),
                            )
                        ot = outpool.tile([P, 512], F32, tag="oto")
                        nc.vector.tensor_add(
                            ot[:], ps[:], bpt[:, jc * 512 : (jc + 1) * 512]
                        )
                        nc.sync.dma_start(
                            out[lt * P : (lt + 1) * P, jc * 512 : (jc + 1) * 512],
                            ot[:],
                        )

    if split_waits:
        _split_multi_waits(nc)
    return nc


def _rope_tables():
    inv = 1.0 / (ROPE_THETA ** (np.arange(0, HD, 2, dtype=np.float32) / HD))
    t = np.arange(T, dtype=np.float32)
    freqs = np.einsum("i,j->ij", t, inv)  # [T, 32]
    freqs = np.repeat(freqs, 2, axis=-1)  # [T, 64]
    cosT = np.cos(freqs).T  # [64, T]
    sinT = np.sin(freqs).T
    cosk = np.tile(cosT, (2, 1)).astype(np.float32)  # [128, T]
    sink = np.tile(sinT, (2, 1)).astype(np.float32)
    return np.ascontiguousarray(cosk), np.ascontiguousarray(sink)


_NC_CACHE = {}


def make_in_maps(x, Wq, Wk, Wv, Wp, bp):
    cosk, sink = _rope_tables()
    bpb = np.ascontiguousarray(np.tile(np.asarray(bp, np.float32)[None, :], (P, 1)))
    bf = lambda a: np.ascontiguousarray(np.asarray(a, np.float32)).astype(
        ml_dtypes.bfloat16
    )
    Wqb, Wkb, Wvb, Wpb = bf(Wq), bf(Wk), bf(Wv), bf(Wp)
    in_maps = []
    for c in range(8):
        b, qh = c // 2, c % 2
        xT = np.asarray(x[b], np.float32).T  # [D, T]
        roll = qh * TQ
        in_maps.append(
            {
                "xT": bf(np.roll(xT, -roll, axis=1)),
                "Wq": Wqb,
                "Wk": Wkb,
                "Wv": Wvb,
                "Wp": Wpb,
                "bpb": bpb,
                "cosk": bf(np.roll(cosk, -roll, axis=1)),
                "sink": bf(np.roll(sink, -roll, axis=1)),
            }
        )
    return in_maps


def kernel(x, h, w, Wq, Wk, Wv, Wp, bp, _trace=False, **trace_kwargs):
    x = np.asarray(x, np.float32)
    in_maps = make_in_maps(x, Wq, Wk, Wv, Wp, bp)
    if "nc" not in _NC_CACHE:
        _NC_CACHE["nc"] = build_nc()
    nc = _NC_CACHE["nc"]
    res = run_bass_kernel_spmd(
        nc, in_maps, list(range(8)), trace=_trace, **trace_kwargs
    )
    out = np.empty((B, T, D), np.float32)
    for c in range(8):
        b, qh = c // 2, c % 2
        out[b, qh * TQ : (qh + 1) * TQ, :] = res.results[c]["out"]
    kernel.last_result = res
    return out


# revision 35
# speedup vs baseline: 1.4847x; 1.0009x over previous
"""AttentionWithRoPE on 8 Trainium2 NeuronCores.

Sharding: batch x query-half -> 8 independent cores (no collectives).
Core c handles batch b=c//2, query rows [qh*1024, (qh+1)*1024) with qh=c%2.
The host rolls the t axis per core so the query rows always sit in columns
[0, TQ) of xT; cosk/sink are rolled identically, so RoPE sees true positions
and the softmax key set is unchanged (order-invariant).

Per-core plan (engine-balanced around the ACT exp street):
  Projections in bf16 (x, Wq/Wk/Wv/Wp host-converted; f32 PSUM accum).
  RoPE: signed pair-swap permutation matmul on PE + DVE combine; Q^T/K^T
  are written as fp8e4 (total quantization noise ~1% on the final
  output, well under the 2e-2 gate).
  S^T per head: ONE fp8 DoubleRow matmul at 0.5 cycles/row -- the d=64
  contraction rides slot 0 (lhsT = K^T with a stride-0 slot broadcast)
  while rhs slot 1 points at a zero strip appended to qtp, so no
  layout fold is needed.
  exp on ACT in [128, 1024] tiles (both heads of a pair share one
  2-bank PSUM tile) -> bf16 P^T ring buffer (RING slots; exp of chunk
  lc overlaps the AV consumption of chunk lc-1).
  AV with l on partitions: out[l, d|1] = P^T-tile.T @ [V|1]-tile, a
  65-column moving operand (65 rows/matmul), accumulated over the 16
  key tiles; softmax row sums ride along as column 64, normalized by
  DVE reciprocal + per-partition tensor_scalar multiply.
  attnout goes bf16 [l, j] -> [j, l] via DMA-engine transposes
  (dma_start_transpose), then out = aotT.T @ Wp + bp in bf16.
  V stays SBUF-resident in bf16 (no DRAM staging); attnout likewise.

Schedule: one "exp street" of 32 units per pair paced by ACT
(~1.04us/unit). Each unit emits S matmuls + the exp; interleaved
closures keep the PE busy: AV streams of the previous query chunk at
units 0..7, the previous pair's transposes at unit 8, and the next
pair's Q/K projection halves plus V-projection tiles at units 8..15 /
24..31. V group g (4 heads) is produced across attentions 2g-1 and 2g,
one half per pair, so every pair carries a near-equal PE load (~32us)
just under the ACT street (~33us).
"""

import sys

sys.path.insert(0, "/opt/trn_rl_repo")

from contextlib import ExitStack

import numpy as np
import ml_dtypes

import concourse.bass as bass
import concourse.mybir as mybir
import concourse.tile as tile
from concourse.bass_utils import run_bass_kernel_spmd

F32 = mybir.dt.float32
F32R = mybir.dt.float32r
BF16 = mybir.dt.bfloat16
F8 = mybir.dt.float8e4
AF = mybir.ActivationFunctionType
MUL = mybir.AluOpType.mult
DR = mybir.MatmulPerfMode.DoubleRow

B, T, D = 4, 2048, 1024
H, HD = 16, 64
P = 128
TQ = 1024  # query rows per core
NT = T // P  # 16 key tiles
DO = D // P  # 8 contraction tiles
RING = 26  # P^T ring slots (16 live + AV-closure lag, see emit_attention)
SCALE = float(D) ** -0.5
ROPE_THETA = 10000.0

_ws_ctr = [0]


def _split_multi_waits(nc):
    """The walrus build in this container accepts at most one sync-wait per
    engine instruction. Hoist all but one wait of each instruction into
    standalone EventSemaphore instructions on the same engine, placed
    immediately before it (engines are in-order, so semantics are identical)."""
    n = 0
    for f in nc.m.functions:
        for blk in f.blocks:
            insts = list(blk.instructions)
            newlist = []
            changed = False
            for inst in insts:
                si = getattr(inst, "sync_info", None)
                waits = list(si.on_wait) if si is not None and si.on_wait else []
                if len(waits) > 1:
                    for w in waits[:-1]:
                        _ws_ctr[0] += 1
                        evs = mybir.InstEventSemaphore(
                            name=f"WSPLIT-{_ws_ctr[0]}", ins=[], outs=[]
                        )
                        evs.engine = inst.engine
                        evs.sync_info = mybir.SyncInfo(on_wait=[w], on_update=[])
                        newlist.append(evs)
                        n += 1
                    inst.sync_info = mybir.SyncInfo(
                        on_wait=[waits[-1]], on_update=list(si.on_update)
                    )
                    changed = True
                newlist.append(inst)
            if changed:
                blk.instructions[:] = newlist
    return n


def _perm_lhsT():
    """lhsT for the rotate-half permutation: out = lhsT.T @ q gives
    out[2i] = -q[2i+1], out[2i+1] = q[2i]."""
    m = np.zeros((P, P), np.float32)
    for i in range(P // 2):
        m[2 * i + 1, 2 * i] = -1.0
        m[2 * i, 2 * i + 1] = 1.0
    return m


def build_nc(split_waits=True, reps=1):
    nc = bass.Bass(trn_type="TRN2", target_bir_lowering=False, debug=False)

    xT = nc.dram_tensor("xT", [D, T], BF16, kind="ExternalInput").ap()
    Wq = nc.dram_tensor("Wq", [D, D], BF16, kind="ExternalInput").ap()
    Wk = nc.dram_tensor("Wk", [D, D], BF16, kind="ExternalInput").ap()
    Wv = nc.dram_tensor("Wv", [D, D], BF16, kind="ExternalInput").ap()
    Wp = nc.dram_tensor("Wp", [D, D], BF16, kind="ExternalInput").ap()
    bpb = nc.dram_tensor("bpb", [P, D], F32, kind="ExternalInput").ap()
    cosk = nc.dram_tensor("cosk", [P, T], BF16, kind="ExternalInput").ap()
    sink = nc.dram_tensor("sink", [P, T], BF16, kind="ExternalInput").ap()
    out = nc.dram_tensor("out", [TQ, D], F32, kind="ExternalOutput").ap()

    permc = nc.inline_tensor(_perm_lhsT(), name="permc").ap()

    xT_r = xT.rearrange("(do dp) t -> dp do t", dp=P)  # [128, 8, 2048]
    Wq_r = Wq.rearrange("(do dp) j -> dp do j", dp=P)
    Wk_r = Wk.rearrange("(do dp) j -> dp do j", dp=P)
    Wv_r = Wv.rearrange("(do dp) j -> dp do j", dp=P)
    Wp_r = Wp.rearrange("(ko kp) j -> kp ko j", kp=P)

    with tile.TileContext(nc) as tc:
      for _rep in range(reps):
        with ExitStack() as top:
            persist = top.enter_context(tc.tile_pool(name="persist", bufs=1))
            permf = persist.tile([P, P], F32R)
            permt = persist.tile([P, P], BF16)
            ck = persist.tile([P, T], BF16)
            sk = persist.tile([P, T], BF16)
            v8 = persist.tile([P, NT, H, 65], BF16)
            p8 = persist.tile([P, RING, 1024], BF16)
            aotT = persist.tile([P, DO, TQ], BF16)

            with tc.tile_pool(name="xpool", bufs=1) as xpool:
                xts = xpool.tile([P, DO, T], BF16)

                with ExitStack() as pmain:
                    wvgpool = pmain.enter_context(tc.tile_pool(name="w_vg", bufs=2))
                    wpool = pmain.enter_context(tc.tile_pool(name="w_qk", bufs=2))
                    tpool = pmain.enter_context(tc.tile_pool(name="t_qk", bufs=2))
                    kqpool = pmain.enter_context(tc.tile_pool(name="kq", bufs=2))
                    pmm = pmain.enter_context(
                        tc.tile_pool(name="ps_qk", bufs=1, space="PSUM")
                    )
                    pmisc = pmain.enter_context(
                        tc.tile_pool(name="ps_misc", bufs=1, space="PSUM")
                    )

                    def proj_rope_halves(w_t, dst, dst_col, src_col):
                        """One [128, 512] Q/K projection tile + RoPE into dst
                        (fp8), split into two closures so the PE burst
                        interleaves finely with the exp street."""
                        ref = {}

                        def ha():
                            ps = pmm.tile([P, 512], F32, tag="ps")
                            ref["ps"] = ps
                            for do in range(4):
                                nc.tensor.matmul(
                                    ps[:],
                                    w_t[:, do],
                                    xts[:, do, src_col * 512 : (src_col + 1) * 512],
                                    start=(do == 0),
                                    stop=False,
                                )

                        def hb():
                            ps = ref["ps"]
                            for do in range(4, DO):
                                nc.tensor.matmul(
                                    ps[:],
                                    w_t[:, do],
                                    xts[:, do, src_col * 512 : (src_col + 1) * 512],
                                    start=False,
                                    stop=(do == DO - 1),
                                )
                            raw = tpool.tile([P, 512], BF16, tag="raw")
                            nc.vector.tensor_copy(raw[:], ps[:])
                            pr = pmisc.tile([P, 512], F32, tag="misc")
                            nc.tensor.matmul(
                                pr[:], permt[:], raw[:], start=True, stop=True
                            )
                            t1 = tpool.tile([P, 512], BF16, tag="t1")
                            nc.vector.tensor_mul(
                                t1[:],
                                raw[:],
                                ck[:, src_col * 512 : (src_col + 1) * 512],
                            )
                            t2 = tpool.tile([P, 512], BF16, tag="t2")
                            nc.vector.tensor_mul(
                                t2[:], pr[:], sk[:, src_col * 512 : (src_col + 1) * 512]
                            )
                            nc.vector.tensor_add(
                                dst[:, dst_col * 512 : (dst_col + 1) * 512],
                                t1[:],
                                t2[:],
                            )

                        return [ha, hb]

                    def emit_proj_jo(jo, preload=False):
                        """Q^T/K^T (fp8) for head pair jo; returns 12 half-step
                        closures: q0a q0b q1a q1b k0a..k3b."""
                        ktp = kqpool.tile([P, T], F8, tag="ktp")
                        qtp = kqpool.tile([P, TQ + 512], F8, tag="qtp")

                        def load_w():
                            wq_t = wpool.tile([P, DO, P], BF16, tag="wq")
                            nc.gpsimd.dma_start(
                                wq_t[:], Wq_r[:, :, jo * P : (jo + 1) * P]
                            )
                            wk_t = wpool.tile([P, DO, P], BF16, tag="wk")
                            nc.gpsimd.dma_start(
                                wk_t[:], Wk_r[:, :, jo * P : (jo + 1) * P]
                            )
                            nc.vector.memset(qtp[:, TQ : TQ + 512], 0.0)
                            return wq_t, wk_t

                        wref = []
                        if preload:
                            wref.extend(load_w())

                        halves = []
                        for i in range(6):
                            ref = {}

                            def mk(i=i, ref=ref):
                                def lazy_a():
                                    if not wref:
                                        wref.extend(load_w())
                                    wq_t, wk_t = wref
                                    if i < 2:
                                        hs = proj_rope_halves(wq_t, qtp, i, i)
                                    else:
                                        hs = proj_rope_halves(
                                            wk_t, ktp, i - 2, i - 2
                                        )
                                    ref["hs"] = hs
                                    hs[0]()

                                def lazy_b():
                                    ref["hs"][1]()

                                return [lazy_a, lazy_b]

                            halves.extend(mk())
                        return ktp, qtp, halves

                    def load_wv(g):
                        wv_t = wvgpool.tile([P, DO, 256], BF16, tag="wvg")
                        nc.gpsimd.dma_start(
                            wv_t[:], Wv_r[:, :, g * 256 : (g + 1) * 256]
                        )
                        return wv_t

                    def v_step(wv_t, g, to):
                        """V projection for head group g (4 heads), key tile to."""
                        ps = pmisc.tile([P, 512], F32, tag="misc", name="psv")
                        for do in range(DO):
                            nc.tensor.matmul(
                                ps[:, 0:256],
                                xts[:, do, to * P : (to + 1) * P],
                                wv_t[:, do],
                                start=(do == 0),
                                stop=(do == DO - 1),
                            )
                        nc.vector.tensor_copy(
                            v8[:, to, 4 * g : 4 * g + 4, 0:64],
                            ps[:, 0:256].rearrange("tp (h e) -> tp h e", e=64),
                        )

                    # ------- phase A: V group 0 (to 0..7) + pair-0 Q/K proj ----
                    with ExitStack() as p1b:
                        pmmv = p1b.enter_context(
                            tc.tile_pool(name="ps_v", bufs=2, space="PSUM")
                        )
                        # PE warm-up: dummy matmuls on a memset tile keep the
                        # tensor engine busy through the initial DMA wait so
                        # the p-state ramp (0.65->2.4GHz over 3us of
                        # continuous execution) completes before real work.
                        wrm = tpool.tile([P, P], BF16, tag="raw", name="warm")
                        nc.vector.memset(wrm[:], 0.0)
                        wps = pmisc.tile([P, 512], F32, tag="misc", name="warmps")
                        for _ in range(12):
                            nc.tensor.matmul(
                                wps[:, 0:P], wrm[:], wrm[:], start=True, stop=True
                            )
                        nc.sync.dma_start(xts[:, :, 0:512], xT_r[:, :, 0:512])
                        wv_g0 = load_wv(0)
                        nc.gpsimd.dma_start(
                            xts[:, :, 512:1024], xT_r[:, :, 512:1024]
                        )
                        ktp0, qtp0, halves0 = emit_proj_jo(0, preload=True)
                        nc.sync.dma_start(ck[:], cosk[:])
                        nc.sync.dma_start(sk[:], sink[:])
                        nc.gpsimd.dma_start(
                            xts[:, :, 1536:2048], xT_r[:, :, 1536:2048]
                        )
                        nc.sync.dma_start(
                            xts[:, :, 1024:1536], xT_r[:, :, 1024:1536]
                        )
                        nc.gpsimd.dma_start(permf[:], permc.bitcast(F32R))
                        nc.vector.tensor_copy(permt[:], permf.bitcast(F32))
                        nc.vector.memset(v8[:, :, :, 64:65], 1.0)

                        def v0_step(to):
                            ps = pmmv.tile([P, 512], F32, tag="psv")
                            for do in range(DO):
                                nc.tensor.matmul(
                                    ps[:, 0:256],
                                    xts[:, do, to * P : (to + 1) * P],
                                    wv_g0[:, do],
                                    start=(do == 0),
                                    stop=(do == DO - 1),
                                )
                            nc.vector.tensor_copy(
                                v8[:, to, 0:4, 0:64],
                                ps[:, 0:256].rearrange("tp (h e) -> tp h e", e=64),
                            )

                        # proj-0: q0 (halves 0,1) + k0..k3 (halves 4..11);
                        # q1 (halves 2,3) is deferred into attention(0).
                        pa_proj = [halves0[i] for i in (0, 1, 4, 5, 6, 7, 8, 9, 10, 11)]
                        # V tiles 0-3 run while the wq/wk loads land; the proj
                        # halves then interleave with the rest.
                        seq = [0, 1, 2, 3, "p0", "p1", "p2", "p3", 4, 5,
                               "p4", "p5", 6, 7, "p6", "p7", "p8", "p9"]
                        for item in seq:
                            if isinstance(item, int):
                                v0_step(item)
                            else:
                                pa_proj[int(item[1:])]()

                    apool = pmain.enter_context(tc.tile_pool(name="aotp", bufs=2))
                    smpool = pmain.enter_context(tc.tile_pool(name="sm", bufs=3))
                    pss = pmain.enter_context(
                        tc.tile_pool(name="ps_s", bufs=2, space="PSUM")
                    )
                    pav = pmain.enter_context(
                        tc.tile_pool(name="ps_av", bufs=2, space="PSUM")
                    )

                    def emit_attention(p, ktp, qtp, av_prev, tr_prev, work):
                        """Attention for head pair p.

                        av_prev: 16 AV-stream closures of pair p-1's second
                        query chunk -- run at this pair's lc0 units 0..7.
                        tr_prev: pair p-1's transpose closure (unit 8).
                        work: proj/V closures for upcoming pairs, spread over
                        units 8..15 of both chunks.
                        Returns (av_lc1_closures, tr_closure) for pair p.
                        """
                        q3 = qtp.rearrange("d (s l) -> d s l", s=3)
                        aotP = apool.tile([P, TQ // P, P], BF16, tag="aotP")

                        def av_stream(lc, h, lt):
                            ltg = lc * 4 + lt
                            pav_t = pav.tile([P, 512], F32, tag="pav")
                            for to in range(NT):
                                s = (p * 32 + lc * NT + to) % RING
                                nc.tensor.matmul(
                                    pav_t[:, 0:65],
                                    p8[
                                        :,
                                        s,
                                        512 * h + lt * P : 512 * h + (lt + 1) * P,
                                    ],
                                    v8[:, to, 2 * p + h, :],
                                    start=(to == 0),
                                    stop=(to == NT - 1),
                                )
                            avs = smpool.tile([P, 65], F32, tag="avs")
                            nc.vector.tensor_copy(avs[:], pav_t[:, 0:65])
                            rc = smpool.tile([P, 1], F32, tag="rc")
                            with nc.allow_low_precision(
                                reason="softmax denom reciprocal"
                            ):
                                nc.vector.reciprocal(rc[:], avs[:, 64:65])
                            nc.vector.tensor_scalar_mul(
                                aotP[:, ltg, 64 * h : 64 * h + 64],
                                avs[:, 0:64],
                                rc[:],
                            )

                        av_cl = {
                            lc: [
                                (lambda lc=lc, h=h, lt=lt: av_stream(lc, h, lt))
                                for h in range(2)
                                for lt in range(4)
                            ]
                            for lc in range(2)
                        }

                        def tr_closure():
                            for ltg in range(TQ // P):
                                nc.sync.dma_start_transpose(
                                    aotT[:, p, ltg * P : (ltg + 1) * P],
                                    aotP[:, ltg, :],
                                )

                        # slot schedule: unit -> closures
                        sched = {u: [] for u in range(32)}
                        if av_prev is not None:
                            for i, cl in enumerate(av_prev):
                                sched[i // 2].append(cl)
                        for i, cl in enumerate(av_cl[0]):
                            sched[16 + i // 2].append(cl)
                        if tr_prev is not None:
                            sched[8].append(tr_prev)
                        wslots = list(range(8, 16)) + list(range(24, 32))
                        if av_prev is None:
                            wslots = list(range(0, 16)) + list(range(24, 32))
                        wq = list(work)
                        ui = 0
                        while wq and ui < len(wslots):
                            u = wslots[ui]
                            item = wq[0]
                            if isinstance(item, tuple) and item[0] == "heavy":
                                if not sched[u]:
                                    sched[u].append(item[1])
                                    wq.pop(0)
                                    ui += 2
                                else:
                                    ui += 1
                                continue
                            if len(sched[u]) < 2:
                                sched[u].append(item)
                                wq.pop(0)
                            else:
                                ui += 1
                        assert not wq, f"work overflow pair {p}: {len(wq)} left"

                        for lc in range(2):
                            for to in range(NT):
                                s = (p * 32 + lc * NT + to) % RING
                                pst = pss.tile([P, 1024], F32, tag="pss")
                                for h in range(2):
                                    r0 = 64 * h
                                    lhsT = (
                                        ktp[r0 : r0 + 64, to * P : (to + 1) * P]
                                        .unsqueeze(1)
                                        .broadcast_to([64, 2, P])
                                    )
                                    if lc == 0:
                                        rhs = q3[r0 : r0 + 64, 0::2, :]
                                    else:
                                        rhs = q3[r0 : r0 + 64, 1:3, :]
                                    nc.tensor.matmul(
                                        pst[:, 512 * h : 512 * h + 512],
                                        lhsT,
                                        rhs,
                                        start=True,
                                        stop=True,
                                        perf_mode=DR,
                                    )
                                nc.scalar.activation(
                                    p8[:, s, :], pst[:], AF.Exp, scale=SCALE
                                )
                                for cl in sched[lc * NT + to]:
                                    cl()
                        return av_cl[1], tr_closure

                    # V group g (heads 4g..4g+3) is produced across attentions
                    # 2g-2 and 2g-1; its consumers are pairs 2g, 2g+1.
                    wv_ref = {}
                    ktp, qtp = ktp0, qtp0
                    av_prev = None
                    tr_prev = None
                    for p in range(DO):
                        work = []
                        if p == 0:
                            work += [halves0[2], halves0[3]]  # q1 of pair 0
                            work += [
                                (lambda to=to: v_step(wv_g0, 0, to))
                                for to in range(8, NT)
                            ]
                        if p >= 1:
                            g = (p + 1) // 2
                            if g <= 3:
                                if p % 2 == 1:
                                    wv_ref[g] = load_wv(g)
                                    rng_to = range(0, 8)
                                else:
                                    rng_to = range(8, NT)
                                wv_t = wv_ref[g]
                                work += [
                                    (lambda to=to, wv_t=wv_t, g=g: v_step(
                                        wv_t, g, to
                                    ))
                                    for to in rng_to
                                ]
                        if p + 1 < DO:
                            nktp, nqtp, nhalves = emit_proj_jo(p + 1)
                            work += nhalves
                        else:
                            nktp = nqtp = None
                        av_prev, tr_prev = emit_attention(
                            p, ktp, qtp, av_prev, tr_prev, work
                        )
                        ktp, qtp = nktp, nqtp
                    # tail: pair 7 second-chunk AV + its transposes
                    for cl in av_prev:
                        cl()
                    tr_prev()

            # ---------------- output projection tail ----------------------
            with ExitStack() as p3:
                wppool = p3.enter_context(tc.tile_pool(name="wp", bufs=1))
                wp_t = wppool.tile([P, DO, D], BF16)
                for ko in range(DO):
                    nc.sync.dma_start(wp_t[:, ko], Wp_r[:, ko])
                bpt = wppool.tile([P, D], F32)
                nc.sync.dma_start(bpt[:], bpb[:])
                outpool = p3.enter_context(tc.tile_pool(name="outp", bufs=3))
                ps_f = p3.enter_context(
                    tc.tile_pool(name="ps_f", bufs=4, space="PSUM")
                )
                for idx in range(16):
                    lt, jc = idx // 2, idx % 2
                    ps = ps_f.tile([P, 512], F32, tag="psf")
                    for ko in range(DO):
                        nc.tensor.matmul(
                            ps[:],
                            aotT[:, ko, lt * P : (lt + 1) * P],
                            wp_t[:, ko, jc * 512 : (jc + 1) * 512],
                            start=(ko == 0),
                            stop=(ko == DO - 1),
                        )
                    ot = outpool.tile([P, 512], F32, tag="oto")
                    nc.vector.tensor_add(
                        ot[:], ps[:], bpt[:, jc * 512 : (jc + 1) * 512]
                    )
                    nc.sync.dma_start(
                        out[lt * P : (lt + 1) * P, jc * 512 : (jc + 1) * 512],
                        ot[:],
                    )

    if split_waits:
        _split_multi_waits(nc)
    return nc


def _rope_tables():
    inv = 1.0 / (ROPE_THETA ** (np.arange(0, HD, 2, dtype=np.float32) / HD))
    t = np.arange(T, dtype=np.float32)
    freqs = np.einsum("i,j->ij", t, inv)  # [T, 32]
    freqs = np.repeat(freqs, 2, axis=-1)  # [T, 64]
    cosT = np.cos(freqs).T  # [64, T]
    sinT = np.sin(freqs).T
    cosk = np.tile(cosT, (2, 1)).astype(np.float32)  # [128, T]
    sink = np.tile(sinT, (2, 1)).astype(np.float32)
    return np.ascontiguousarray(cosk), np.ascontiguousarray(sink)


_NC_CACHE = {}


def make_in_maps(x, Wq, Wk, Wv, Wp, bp):
    cosk, sink = _rope_tables()
    bpb = np.ascontiguousarray(np.tile(np.asarray(bp, np.float32)[None, :], (P, 1)))
    bf = lambda a: np.ascontiguousarray(np.asarray(a, np.float32)).astype(
        ml_dtypes.bfloat16
    )
    Wqb, Wkb, Wvb, Wpb = bf(Wq), bf(Wk), bf(Wv), bf(Wp)
    in_maps = []
    for c in range(8):
        b, qh = c // 2, c % 2
        xT = np.asarray(x[b], np.float32).T  # [D, T]
        roll = qh * TQ
        in_maps.append(
            {
                "xT": bf(np.roll(xT, -roll, axis=1)),
                "Wq": Wqb,
                "Wk": Wkb,
                "Wv": Wvb,
                "Wp": Wpb,
                "bpb": bpb,
                "cosk": bf(np.roll(cosk, -roll, axis=1)),
                "sink": bf(np.roll(sink, -roll, axis=1)),
            }
        )
    return in_maps


def kernel(x, h, w, Wq, Wk, Wv, Wp, bp, _trace=False, **trace_kwargs):
    x = np.asarray(x, np.float32)
    in_maps = make_in_maps(x, Wq, Wk, Wv, Wp, bp)
    if "nc" not in _NC_CACHE:
        _NC_CACHE["nc"] = build_nc()
    nc = _NC_CACHE["nc"]
    res = run_bass_kernel_spmd(
        nc, in_maps, list(range(8)), trace=_trace, **trace_kwargs
    )
    out = np.empty((B, T, D), np.float32)
    for c in range(8):
        b, qh = c // 2, c % 2
        out[b, qh * TQ : (qh + 1) * TQ, :] = res.results[c]["out"]
    kernel.last_result = res
    return out


# revision 38
# speedup vs baseline: 1.5051x; 1.0137x over previous
"""AttentionWithRoPE on 8 Trainium2 NeuronCores.

Sharding: batch x query-half -> 8 independent cores (no collectives).
Core c handles batch b=c//2, query rows [qh*1024, (qh+1)*1024) with qh=c%2.
The host rolls the t axis per core so the query rows always sit in columns
[0, TQ) of xT; cosk/sink are rolled identically, so RoPE sees true positions
and the softmax key set is unchanged (order-invariant).

Per-core plan (engine-balanced around the ACT exp street):
  Projections in bf16 (x, Wq/Wk/Wv/Wp host-converted; f32 PSUM accum).
  RoPE: rotate-half via a DVE stream_shuffle (adjacent-partition swap,
  sign folded into the host sin table) + DVE combine; Q^T/K^T are
  written as fp8e4 (total quantization noise ~1% on the final output,
  well under the 2e-2 gate).
  S^T per head: ONE fp8 DoubleRow matmul at 0.5 cycles/row -- the d=64
  contraction rides slot 0 (lhsT = K^T with a stride-0 slot broadcast)
  while rhs slot 1 points at a zero strip appended to qtp, so no
  layout fold is needed.
  exp on ACT in [128, 1024] tiles (both heads of a pair share one
  2-bank PSUM tile) -> bf16 P^T ring buffer (RING slots; exp of chunk
  lc overlaps the AV consumption of chunk lc-1).
  AV with l on partitions: out[l, d|1] = P^T-tile.T @ [V|1]-tile, a
  65-column moving operand (65 rows/matmul), accumulated over the 16
  key tiles; softmax row sums ride along as column 64, normalized by
  DVE reciprocal + per-partition tensor_scalar multiply.
  attnout goes bf16 [l, j] -> [j, l] via DMA-engine transposes
  (dma_start_transpose), then out = aotT.T @ Wp + bp in bf16.
  V stays SBUF-resident in bf16 (no DRAM staging); attnout likewise.

Schedule: one "exp street" of 32 units per pair paced by ACT
(~1.04us/unit). Each unit emits S matmuls + the exp; interleaved
closures keep the PE busy: AV streams of the previous query chunk at
units 0..7, the previous pair's transposes at unit 8, and the next
pair's Q/K projection halves plus V-projection tiles at units 8..15 /
24..31. V group g (4 heads) is produced across attentions 2g-1 and 2g,
one half per pair, so every pair carries a near-equal PE load (~32us)
just under the ACT street (~33us).
"""

import sys

sys.path.insert(0, "/opt/trn_rl_repo")

from contextlib import ExitStack

import numpy as np
import ml_dtypes

import concourse.bass as bass
import concourse.mybir as mybir
import concourse.tile as tile
from concourse.bass_utils import run_bass_kernel_spmd

F32 = mybir.dt.float32
F32R = mybir.dt.float32r
BF16 = mybir.dt.bfloat16
F8 = mybir.dt.float8e4
AF = mybir.ActivationFunctionType
MUL = mybir.AluOpType.mult
DR = mybir.MatmulPerfMode.DoubleRow

B, T, D = 4, 2048, 1024
H, HD = 16, 64
P = 128
TQ = 1024  # query rows per core
NT = T // P  # 16 key tiles
DO = D // P  # 8 contraction tiles
RING = 26  # P^T ring slots (16 live + AV-closure lag, see emit_attention)
SCALE = float(D) ** -0.5
ROPE_THETA = 10000.0
SHUF_MASK = [i ^ 1 for i in range(32)]  # adjacent-partition swap (per 32-lane group)

_ws_ctr = [0]


def _split_multi_waits(nc):
    """The walrus build in this container accepts at most one sync-wait per
    engine instruction. Hoist all but one wait of each instruction into
    standalone EventSemaphore instructions on the same engine, placed
    immediately before it (engines are in-order, so semantics are identical)."""
    n = 0
    for f in nc.m.functions:
        for blk in f.blocks:
            insts = list(blk.instructions)
            newlist = []
            changed = False
            for inst in insts:
                si = getattr(inst, "sync_info", None)
                waits = list(si.on_wait) if si is not None and si.on_wait else []
                if len(waits) > 1:
                    for w in waits[:-1]:
                        _ws_ctr[0] += 1
                        evs = mybir.InstEventSemaphore(
                            name=f"WSPLIT-{_ws_ctr[0]}", ins=[], outs=[]
                        )
                        evs.engine = inst.engine
                        evs.sync_info = mybir.SyncInfo(on_wait=[w], on_update=[])
                        newlist.append(evs)
                        n += 1
                    inst.sync_info = mybir.SyncInfo(
                        on_wait=[waits[-1]], on_update=list(si.on_update)
                    )
                    changed = True
                newlist.append(inst)
            if changed:
                blk.instructions[:] = newlist
    return n


def build_nc(split_waits=True, reps=1):
    nc = bass.Bass(trn_type="TRN2", target_bir_lowering=False, debug=False)

    xT = nc.dram_tensor("xT", [D, T], BF16, kind="ExternalInput").ap()
    Wq = nc.dram_tensor("Wq", [D, D], BF16, kind="ExternalInput").ap()
    Wk = nc.dram_tensor("Wk", [D, D], BF16, kind="ExternalInput").ap()
    Wv = nc.dram_tensor("Wv", [D, D], BF16, kind="ExternalInput").ap()
    Wp = nc.dram_tensor("Wp", [D, D], BF16, kind="ExternalInput").ap()
    bpb = nc.dram_tensor("bpb", [P, D], F32, kind="ExternalInput").ap()
    cosk = nc.dram_tensor("cosk", [P, T], BF16, kind="ExternalInput").ap()
    sink = nc.dram_tensor("sink", [P, T], BF16, kind="ExternalInput").ap()
    out = nc.dram_tensor("out", [TQ, D], F32, kind="ExternalOutput").ap()

    xT_r = xT.rearrange("(do dp) t -> dp do t", dp=P)  # [128, 8, 2048]
    Wq_r = Wq.rearrange("(do dp) j -> dp do j", dp=P)
    Wk_r = Wk.rearrange("(do dp) j -> dp do j", dp=P)
    Wv_r = Wv.rearrange("(do dp) j -> dp do j", dp=P)
    Wp_r = Wp.rearrange("(ko kp) j -> kp ko j", kp=P)

    with tile.TileContext(nc) as tc:
      for _rep in range(reps):
        with ExitStack() as top:
            persist = top.enter_context(tc.tile_pool(name="persist", bufs=1))
            ck = persist.tile([P, T], BF16)
            sk = persist.tile([P, T], BF16)
            v8 = persist.tile([P, NT, H, 65], BF16)
            p8 = persist.tile([P, RING, 1024], BF16)
            aotT = persist.tile([P, DO, TQ], BF16)

            with tc.tile_pool(name="xpool", bufs=1) as xpool:
                xts = xpool.tile([P, DO, T], BF16)

                with ExitStack() as pmain:
                    wvgpool = pmain.enter_context(tc.tile_pool(name="w_vg", bufs=2))
                    wpool = pmain.enter_context(tc.tile_pool(name="w_qk", bufs=2))
                    tpool = pmain.enter_context(tc.tile_pool(name="t_qk", bufs=2))
                    kqpool = pmain.enter_context(tc.tile_pool(name="kq", bufs=2))
                    pmm = pmain.enter_context(
                        tc.tile_pool(name="ps_qk", bufs=1, space="PSUM")
                    )
                    pmisc = pmain.enter_context(
                        tc.tile_pool(name="ps_misc", bufs=1, space="PSUM")
                    )

                    def proj_rope_halves(w_t, dst, dst_col, src_col):
                        """One [128, 512] Q/K projection tile + RoPE into dst
                        (fp8), split into two closures so the PE burst
                        interleaves finely with the exp street."""
                        ref = {}

                        def ha():
                            ps = pmm.tile([P, 512], F32, tag="ps")
                            ref["ps"] = ps
                            for do in range(4):
                                nc.tensor.matmul(
                                    ps[:],
                                    w_t[:, do],
                                    xts[:, do, src_col * 512 : (src_col + 1) * 512],
                                    start=(do == 0),
                                    stop=False,
                                )

                        def hb():
                            ps = ref["ps"]
                            for do in range(4, DO):
                                nc.tensor.matmul(
                                    ps[:],
                                    w_t[:, do],
                                    xts[:, do, src_col * 512 : (src_col + 1) * 512],
                                    start=False,
                                    stop=(do == DO - 1),
                                )
                            raw = tpool.tile([P, 512], BF16, tag="raw")
                            nc.vector.tensor_copy(raw[:], ps[:])
                            shuf = tpool.tile([P, 512], BF16, tag="t2s")
                            nc.vector.stream_shuffle(shuf[:], raw[:], SHUF_MASK)
                            t1 = tpool.tile([P, 512], BF16, tag="t1")
                            nc.vector.tensor_mul(
                                t1[:],
                                raw[:],
                                ck[:, src_col * 512 : (src_col + 1) * 512],
                            )
                            t2 = tpool.tile([P, 512], BF16, tag="t2")
                            nc.vector.tensor_mul(
                                t2[:],
                                shuf[:],
                                sk[:, src_col * 512 : (src_col + 1) * 512],
                            )
                            nc.vector.tensor_add(
                                dst[:, dst_col * 512 : (dst_col + 1) * 512],
                                t1[:],
                                t2[:],
                            )

                        return [ha, hb]

                    def emit_proj_jo(jo, preload=False):
                        """Q^T/K^T (fp8) for head pair jo; returns 12 half-step
                        closures: q0a q0b q1a q1b k0a..k3b."""
                        ktp = kqpool.tile([P, T], F8, tag="ktp")
                        qtp = kqpool.tile([P, TQ + 512], F8, tag="qtp")

                        def load_w():
                            wq_t = wpool.tile([P, DO, P], BF16, tag="wq")
                            nc.gpsimd.dma_start(
                                wq_t[:], Wq_r[:, :, jo * P : (jo + 1) * P]
                            )
                            wk_t = wpool.tile([P, DO, P], BF16, tag="wk")
                            nc.gpsimd.dma_start(
                                wk_t[:], Wk_r[:, :, jo * P : (jo + 1) * P]
                            )
                            nc.vector.memset(qtp[:, TQ : TQ + 512], 0.0)
                            return wq_t, wk_t

                        wref = []
                        if preload:
                            wref.extend(load_w())

                        halves = []
                        for i in range(6):
                            ref = {}

                            def mk(i=i, ref=ref):
                                def lazy_a():
                                    if not wref:
                                        wref.extend(load_w())
                                    wq_t, wk_t = wref
                                    if i < 2:
                                        hs = proj_rope_halves(wq_t, qtp, i, i)
                                    else:
                                        hs = proj_rope_halves(
                                            wk_t, ktp, i - 2, i - 2
                                        )
                                    ref["hs"] = hs
                                    hs[0]()

                                def lazy_b():
                                    ref["hs"][1]()

                                return [lazy_a, lazy_b]

                            halves.extend(mk())
                        return ktp, qtp, halves

                    def load_wv(g):
                        wv_t = wvgpool.tile([P, DO, 256], BF16, tag="wvg")
                        nc.gpsimd.dma_start(
                            wv_t[:], Wv_r[:, :, g * 256 : (g + 1) * 256]
                        )
                        return wv_t

                    def v_step(wv_t, g, to):
                        """V projection for head group g (4 heads), key tile to."""
                        ps = pmisc.tile([P, 512], F32, tag="misc", name="psv")
                        for do in range(DO):
                            nc.tensor.matmul(
                                ps[:, 0:256],
                                xts[:, do, to * P : (to + 1) * P],
                                wv_t[:, do],
                                start=(do == 0),
                                stop=(do == DO - 1),
                            )
                        nc.vector.tensor_copy(
                            v8[:, to, 4 * g : 4 * g + 4, 0:64],
                            ps[:, 0:256].rearrange("tp (h e) -> tp h e", e=64),
                        )

                    # ------- phase A: V group 0 (to 0..7) + pair-0 Q/K proj ----
                    with ExitStack() as p1b:
                        pmmv = p1b.enter_context(
                            tc.tile_pool(name="ps_v", bufs=2, space="PSUM")
                        )
                        # PE warm-up: dummy matmuls on a memset tile keep the
                        # tensor engine busy through the initial DMA wait so
                        # the p-state ramp (0.65->2.4GHz over 3us of
                        # continuous execution) completes before real work.
                        wrm = tpool.tile([P, P], BF16, tag="raw", name="warm")
                        nc.vector.memset(wrm[:], 0.0)
                        wps = pmisc.tile([P, 512], F32, tag="misc", name="warmps")
                        for _ in range(12):
                            nc.tensor.matmul(
                                wps[:, 0:P], wrm[:], wrm[:], start=True, stop=True
                            )
                        nc.sync.dma_start(xts[:, :, 0:512], xT_r[:, :, 0:512])
                        wv_g0 = load_wv(0)
                        nc.gpsimd.dma_start(
                            xts[:, :, 512:1024], xT_r[:, :, 512:1024]
                        )
                        ktp0, qtp0, halves0 = emit_proj_jo(0, preload=True)
                        nc.sync.dma_start(ck[:], cosk[:])
                        nc.sync.dma_start(sk[:], sink[:])
                        nc.gpsimd.dma_start(
                            xts[:, :, 1536:2048], xT_r[:, :, 1536:2048]
                        )
                        nc.sync.dma_start(
                            xts[:, :, 1024:1536], xT_r[:, :, 1024:1536]
                        )
                        nc.vector.memset(v8[:, :, :, 64:65], 1.0)

                        def v0_step(to):
                            ps = pmmv.tile([P, 512], F32, tag="psv")
                            for do in range(DO):
                                nc.tensor.matmul(
                                    ps[:, 0:256],
                                    xts[:, do, to * P : (to + 1) * P],
                                    wv_g0[:, do],
                                    start=(do == 0),
                                    stop=(do == DO - 1),
                                )
                            nc.vector.tensor_copy(
                                v8[:, to, 0:4, 0:64],
                                ps[:, 0:256].rearrange("tp (h e) -> tp h e", e=64),
                            )

                        # proj-0: q0 (halves 0,1) + k0..k3 (halves 4..11);
                        # q1 (halves 2,3) is deferred into attention(0).
                        pa_proj = [halves0[i] for i in (0, 1, 4, 5, 6, 7, 8, 9, 10, 11)]
                        # V tiles 0-3 run while the wq/wk loads land; the proj
                        # halves then interleave with the rest.
                        seq = [0, 1, 2, 3, "p0", "p1", "p2", "p3", 4, 5,
                               "p4", "p5", 6, 7, "p6", "p7", "p8", "p9"]
                        for item in seq:
                            if isinstance(item, int):
                                v0_step(item)
                            else:
                                pa_proj[int(item[1:])]()

                    apool = pmain.enter_context(tc.tile_pool(name="aotp", bufs=2))
                    smpool = pmain.enter_context(tc.tile_pool(name="sm", bufs=3))
                    pss = pmain.enter_context(
                        tc.tile_pool(name="ps_s", bufs=2, space="PSUM")
                    )
                    pav = pmain.enter_context(
                        tc.tile_pool(name="ps_av", bufs=2, space="PSUM")
                    )

                    def emit_attention(p, ktp, qtp, av_prev, tr_prev, work):
                        """Attention for head pair p.

                        av_prev: 16 AV-stream closures of pair p-1's second
                        query chunk -- run at this pair's lc0 units 0..7.
                        tr_prev: pair p-1's transpose closure (unit 8).
                        work: proj/V closures for upcoming pairs, spread over
                        units 8..15 of both chunks.
                        Returns (av_lc1_closures, tr_closure) for pair p.
                        """
                        q3 = qtp.rearrange("d (s l) -> d s l", s=3)
                        aotP = apool.tile([P, TQ // P, P], BF16, tag="aotP")

                        def av_stream(lc, h, lt):
                            ltg = lc * 4 + lt
                            pav_t = pav.tile([P, 512], F32, tag="pav")
                            for to in range(NT):
                                s = (p * 32 + lc * NT + to) % RING
                                nc.tensor.matmul(
                                    pav_t[:, 0:65],
                                    p8[
                                        :,
                                        s,
                                        512 * h + lt * P : 512 * h + (lt + 1) * P,
                                    ],
                                    v8[:, to, 2 * p + h, :],
                                    start=(to == 0),
                                    stop=(to == NT - 1),
                                )
                            avs = smpool.tile([P, 65], F32, tag="avs")
                            nc.vector.tensor_copy(avs[:], pav_t[:, 0:65])
                            rc = smpool.tile([P, 1], F32, tag="rc")
                            with nc.allow_low_precision(
                                reason="softmax denom reciprocal"
                            ):
                                nc.vector.reciprocal(rc[:], avs[:, 64:65])
                            nc.vector.tensor_scalar_mul(
                                aotP[:, ltg, 64 * h : 64 * h + 64],
                                avs[:, 0:64],
                                rc[:],
                            )

                        av_cl = {
                            lc: [
                                (lambda lc=lc, h=h, lt=lt: av_stream(lc, h, lt))
                                for h in range(2)
                                for lt in range(4)
                            ]
                            for lc in range(2)
                        }

                        def tr_closure():
                            for ltg in range(TQ // P):
                                nc.sync.dma_start_transpose(
                                    aotT[:, p, ltg * P : (ltg + 1) * P],
                                    aotP[:, ltg, :],
                                )

                        # slot schedule: unit -> closures
                        sched = {u: [] for u in range(32)}
                        if av_prev is not None:
                            for i, cl in enumerate(av_prev):
                                sched[i // 2].append(cl)
                        for i, cl in enumerate(av_cl[0]):
                            sched[16 + i // 2].append(cl)
                        if tr_prev is not None:
                            sched[8].append(tr_prev)
                        wslots = list(range(8, 16)) + list(range(24, 32))
                        if av_prev is None:
                            wslots = list(range(0, 16)) + list(range(24, 32))
                        wq = list(work)
                        ui = 0
                        while wq and ui < len(wslots):
                            u = wslots[ui]
                            item = wq[0]
                            if isinstance(item, tuple) and item[0] == "heavy":
                                if not sched[u]:
                                    sched[u].append(item[1])
                                    wq.pop(0)
                                    ui += 2
                                else:
                                    ui += 1
                                continue
                            if len(sched[u]) < 2:
                                sched[u].append(item)
                                wq.pop(0)
                            else:
                                ui += 1
                        assert not wq, f"work overflow pair {p}: {len(wq)} left"

                        for lc in range(2):
                            for to in range(NT):
                                s = (p * 32 + lc * NT + to) % RING
                                pst = pss.tile([P, 1024], F32, tag="pss")
                                for h in range(2):
                                    r0 = 64 * h
                                    lhsT = (
                                        ktp[r0 : r0 + 64, to * P : (to + 1) * P]
                                        .unsqueeze(1)
                                        .broadcast_to([64, 2, P])
                                    )
                                    if lc == 0:
                                        rhs = q3[r0 : r0 + 64, 0::2, :]
                                    else:
                                        rhs = q3[r0 : r0 + 64, 1:3, :]
                                    nc.tensor.matmul(
                                        pst[:, 512 * h : 512 * h + 512],
                                        lhsT,
                                        rhs,
                                        start=True,
                                        stop=True,
                                        perf_mode=DR,
                                    )
                                nc.scalar.activation(
                                    p8[:, s, :], pst[:], AF.Exp, scale=SCALE
                                )
                                for cl in sched[lc * NT + to]:
                                    cl()
                        return av_cl[1], tr_closure

                    # V group g (heads 4g..4g+3) is produced across attentions
                    # 2g-2 and 2g-1; its consumers are pairs 2g, 2g+1.
                    wv_ref = {}
                    ktp, qtp = ktp0, qtp0
                    av_prev = None
                    tr_prev = None
                    for p in range(DO):
                        work = []
                        if p == 0:
                            work += [halves0[2], halves0[3]]  # q1 of pair 0
                            work += [
                                (lambda to=to: v_step(wv_g0, 0, to))
                                for to in range(8, NT)
                            ]
                        if p >= 1:
                            g = (p + 1) // 2
                            if g <= 3:
                                if p % 2 == 1:
                                    wv_ref[g] = load_wv(g)
                                    rng_to = range(0, 8)
                                else:
                                    rng_to = range(8, NT)
                                wv_t = wv_ref[g]
                                work += [
                                    (lambda to=to, wv_t=wv_t, g=g: v_step(
                                        wv_t, g, to
                                    ))
                                    for to in rng_to
                                ]
                        if p + 1 < DO:
                            nktp, nqtp, nhalves = emit_proj_jo(p + 1)
                            work += nhalves
                        else:
                            nktp = nqtp = None
                        av_prev, tr_prev = emit_attention(
                            p, ktp, qtp, av_prev, tr_prev, work
                        )
                        ktp, qtp = nktp, nqtp
                    # tail: pair 7 second-chunk AV + its transposes
                    for cl in av_prev:
                        cl()
                    tr_prev()

            # ---------------- output projection tail ----------------------
            with ExitStack() as p3:
                wppool = p3.enter_context(tc.tile_pool(name="wp", bufs=1))
                wp_t = wppool.tile([P, DO, D], BF16)
                for ko in range(DO):
                    nc.sync.dma_start(wp_t[:, ko], Wp_r[:, ko])
                bpt = wppool.tile([P, D], F32)
                nc.sync.dma_start(bpt[:], bpb[:])
                outpool = p3.enter_context(tc.tile_pool(name="outp", bufs=3))
                ps_f = p3.enter_context(
                    tc.tile_pool(name="ps_f", bufs=4, space="PSUM")
                )
                for idx in range(16):
                    lt, jc = idx // 2, idx % 2
                    ps = ps_f.tile([P, 512], F32, tag="psf")
                    for ko in range(DO):
                        nc.tensor.matmul(
                            ps[:],
                            aotT[:, ko, lt * P : (lt + 1) * P],
                            wp_t[:, ko, jc * 512 : (jc + 1) * 512],
                            start=(ko == 0),
                            stop=(ko == DO - 1),
                        )
                    ot = outpool.tile([P, 512], F32, tag="oto")
                    nc.vector.tensor_add(
                        ot[:], ps[:], bpt[:, jc * 512 : (jc + 1) * 512]
                    )
                    nc.sync.dma_start(
                        out[lt * P : (lt + 1) * P, jc * 512 : (jc + 1) * 512],
                        ot[:],
                    )

    if split_waits:
        _split_multi_waits(nc)
    return nc


def _rope_tables():
    inv = 1.0 / (ROPE_THETA ** (np.arange(0, HD, 2, dtype=np.float32) / HD))
    t = np.arange(T, dtype=np.float32)
    freqs = np.einsum("i,j->ij", t, inv)  # [T, 32]
    freqs = np.repeat(freqs, 2, axis=-1)  # [T, 64]
    cosT = np.cos(freqs).T  # [64, T]
    sinT = np.sin(freqs).T
    cosk = np.tile(cosT, (2, 1)).astype(np.float32)  # [128, T]
    sink = np.tile(sinT, (2, 1)).astype(np.float32)
    # sign-fold for the stream_shuffle rotate-half: rot(x)[2i] = -x[2i+1]
    sink[0::2] *= -1.0
    return np.ascontiguousarray(cosk), np.ascontiguousarray(sink)


_NC_CACHE = {}


def make_in_maps(x, Wq, Wk, Wv, Wp, bp):
    cosk, sink = _rope_tables()
    bpb = np.ascontiguousarray(np.tile(np.asarray(bp, np.float32)[None, :], (P, 1)))
    bf = lambda a: np.ascontiguousarray(np.asarray(a, np.float32)).astype(
        ml_dtypes.bfloat16
    )
    Wqb, Wkb, Wvb, Wpb = bf(Wq), bf(Wk), bf(Wv), bf(Wp)
    in_maps = []
    for c in range(8):
        b, qh = c // 2, c % 2
        xT = np.asarray(x[b], np.float32).T  # [D, T]
        roll = qh * TQ
        in_maps.append(
            {
                "xT": bf(np.roll(xT, -roll, axis=1)),
                "Wq": Wqb,
                "Wk": Wkb,
                "Wv": Wvb,
                "Wp": Wpb,
                "bpb": bpb,
                "cosk": bf(np.roll(cosk, -roll, axis=1)),
                "sink": bf(np.roll(sink, -roll, axis=1)),
            }
        )
    return in_maps


def kernel(x, h, w, Wq, Wk, Wv, Wp, bp, _trace=False, **trace_kwargs):
    x = np.asarray(x, np.float32)
    in_maps = make_in_maps(x, Wq, Wk, Wv, Wp, bp)
    if "nc" not in _NC_CACHE:
        _NC_CACHE["nc"] = build_nc()
    nc = _NC_CACHE["nc"]
    res = run_bass_kernel_spmd(
        nc, in_maps, list(range(8)), trace=_trace, **trace_kwargs
    )
    out = np.empty((B, T, D), np.float32)
    for c in range(8):
        b, qh = c // 2, c % 2
        out[b, qh * TQ : (qh + 1) * TQ, :] = res.results[c]["out"]
    kernel.last_result = res
    return out


# revision 44
# speedup vs baseline: 1.5102x; 1.0034x over previous
"""AttentionWithRoPE on 8 Trainium2 NeuronCores.

Sharding: batch x query-half -> 8 independent cores (no collectives).
Core c handles batch b=c//2, query rows [qh*1024, (qh+1)*1024) with qh=c%2.
The host rolls the t axis per core so the query rows always sit in columns
[0, TQ) of xT; cosk/sink are rolled identically, so RoPE sees true positions
and the softmax key set is unchanged (order-invariant).

Per-core plan (engine-balanced around the ACT exp street):
  Projections in bf16 (x, Wq/Wk/Wv/Wp host-converted; f32 PSUM accum).
  RoPE: signed pair-swap permutation matmul on PE + DVE combine; Q^T/K^T
  are written as fp8e4 (total quantization noise ~1% on the final
  output, well under the 2e-2 gate).
  S^T per head: ONE fp8 DoubleRow matmul at 0.5 cycles/row -- the d=64
  contraction rides slot 0 (lhsT = K^T with a stride-0 slot broadcast)
  while rhs slot 1 points at a zero strip appended to qtp, so no
  layout fold is needed.
  exp on ACT in [128, 1024] tiles (both heads of a pair share one
  2-bank PSUM tile) -> bf16 P^T ring buffer (RING slots; exp of chunk
  lc overlaps the AV consumption of chunk lc-1).
  AV with l on partitions: out[l, d|1] = P^T-tile.T @ [V|1]-tile, a
  65-column moving operand (65 rows/matmul), accumulated over the 16
  key tiles; softmax row sums ride along as column 64, normalized by
  DVE reciprocal + per-partition tensor_scalar multiply.
  attnout goes bf16 [l, j] -> [j, l] via DMA-engine transposes
  (dma_start_transpose), then out = aotT.T @ Wp + bp in bf16.
  V stays SBUF-resident in bf16 (no DRAM staging); attnout likewise.

Schedule: one "exp street" of 32 units per pair paced by ACT
(~1.04us/unit). Each unit emits S matmuls + the exp; interleaved
closures keep the PE busy: AV streams of the previous query chunk at
units 0..7, the previous pair's transposes at unit 8, and the next
pair's Q/K projection halves plus V-projection tiles at units 8..15 /
24..31. V group g (4 heads) is produced across attentions 2g-1 and 2g,
one half per pair, so every pair carries a near-equal PE load (~32us)
just under the ACT street (~33us).
"""

import sys

sys.path.insert(0, "/opt/trn_rl_repo")

from contextlib import ExitStack

import numpy as np
import ml_dtypes

import concourse.bass as bass
import concourse.mybir as mybir
import concourse.tile as tile
from concourse.bass_utils import run_bass_kernel_spmd

F32 = mybir.dt.float32
F32R = mybir.dt.float32r
BF16 = mybir.dt.bfloat16
F8 = mybir.dt.float8e4
AF = mybir.ActivationFunctionType
MUL = mybir.AluOpType.mult
DR = mybir.MatmulPerfMode.DoubleRow

B, T, D = 4, 2048, 1024
H, HD = 16, 64
P = 128
TQ = 1024  # query rows per core
NT = T // P  # 16 key tiles
DO = D // P  # 8 contraction tiles
RING = 26  # P^T ring slots (16 live + AV-closure lag, see emit_attention)
SCALE = float(D) ** -0.5
ROPE_THETA = 10000.0
SHUF_MASK = [i ^ 1 for i in range(32)]  # adjacent-partition swap (per 32-lane group)

_ws_ctr = [0]


def _split_multi_waits(nc):
    """The walrus build in this container accepts at most one sync-wait per
    engine instruction. Hoist all but one wait of each instruction into
    standalone EventSemaphore instructions on the same engine, placed
    immediately before it (engines are in-order, so semantics are identical)."""
    n = 0
    for f in nc.m.functions:
        for blk in f.blocks:
            insts = list(blk.instructions)
            newlist = []
            changed = False
            for inst in insts:
                si = getattr(inst, "sync_info", None)
                waits = list(si.on_wait) if si is not None and si.on_wait else []
                if len(waits) > 1:
                    for w in waits[:-1]:
                        _ws_ctr[0] += 1
                        evs = mybir.InstEventSemaphore(
                            name=f"WSPLIT-{_ws_ctr[0]}", ins=[], outs=[]
                        )
                        evs.engine = inst.engine
                        evs.sync_info = mybir.SyncInfo(on_wait=[w], on_update=[])
                        newlist.append(evs)
                        n += 1
                    inst.sync_info = mybir.SyncInfo(
                        on_wait=[waits[-1]], on_update=list(si.on_update)
                    )
                    changed = True
                newlist.append(inst)
            if changed:
                blk.instructions[:] = newlist
    return n


def build_nc(split_waits=True, reps=1):
    nc = bass.Bass(trn_type="TRN2", target_bir_lowering=False, debug=False)

    xT = nc.dram_tensor("xT", [D, T], BF16, kind="ExternalInput").ap()
    Wq = nc.dram_tensor("Wq", [D, D], BF16, kind="ExternalInput").ap()
    Wk = nc.dram_tensor("Wk", [D, D], BF16, kind="ExternalInput").ap()
    Wv = nc.dram_tensor("Wv", [D, D], BF16, kind="ExternalInput").ap()
    Wp = nc.dram_tensor("Wp", [D, D], BF16, kind="ExternalInput").ap()
    bpb = nc.dram_tensor("bpb", [P, D], F32, kind="ExternalInput").ap()
    cosk = nc.dram_tensor("cosk", [P, T], BF16, kind="ExternalInput").ap()
    sink = nc.dram_tensor("sink", [P, T], BF16, kind="ExternalInput").ap()
    out = nc.dram_tensor("out", [TQ, D], F32, kind="ExternalOutput").ap()

    xT_r = xT.rearrange("(do dp) t -> dp do t", dp=P)  # [128, 8, 2048]
    Wq_r = Wq.rearrange("(do dp) j -> dp do j", dp=P)
    Wk_r = Wk.rearrange("(do dp) j -> dp do j", dp=P)
    Wv_r = Wv.rearrange("(do dp) j -> dp do j", dp=P)
    Wp_r = Wp.rearrange("(ko kp) j -> kp ko j", kp=P)

    with tile.TileContext(nc) as tc:
      for _rep in range(reps):
        with ExitStack() as top:
            persist = top.enter_context(tc.tile_pool(name="persist", bufs=1))
            ck = persist.tile([P, T], BF16)
            sk = persist.tile([P, T], BF16)
            v8 = persist.tile([P, NT, H, 65], BF16)
            p8 = persist.tile([P, RING, 1024], BF16)
            aotT = persist.tile([P, DO, TQ], BF16)

            with tc.tile_pool(name="xpool", bufs=1) as xpool:
                xts = xpool.tile([P, DO, T], BF16)

                with ExitStack() as pmain:
                    wvgpool = pmain.enter_context(tc.tile_pool(name="w_vg", bufs=2))
                    wpool = pmain.enter_context(tc.tile_pool(name="w_qk", bufs=2))
                    tpool = pmain.enter_context(tc.tile_pool(name="t_qk", bufs=2))
                    kqpool = pmain.enter_context(tc.tile_pool(name="kq", bufs=2))
                    pmm = pmain.enter_context(
                        tc.tile_pool(name="ps_qk", bufs=1, space="PSUM")
                    )
                    pmisc = pmain.enter_context(
                        tc.tile_pool(name="ps_misc", bufs=1, space="PSUM")
                    )

                    def proj_rope_halves(w_t, dst, dst_col, src_col):
                        """One [128, 512] Q/K projection tile + RoPE into dst
                        (fp8), split into two closures so the PE burst
                        interleaves finely with the exp street."""
                        ref = {}

                        def ha():
                            ps = pmm.tile([P, 512], F32, tag="ps")
                            ref["ps"] = ps
                            for do in range(4):
                                nc.tensor.matmul(
                                    ps[:],
                                    w_t[:, do],
                                    xts[:, do, src_col * 512 : (src_col + 1) * 512],
                                    start=(do == 0),
                                    stop=False,
                                )

                        def hb():
                            ps = ref["ps"]
                            for do in range(4, DO):
                                nc.tensor.matmul(
                                    ps[:],
                                    w_t[:, do],
                                    xts[:, do, src_col * 512 : (src_col + 1) * 512],
                                    start=False,
                                    stop=(do == DO - 1),
                                )
                            raw = tpool.tile([P, 512], BF16, tag="raw")
                            nc.vector.tensor_copy(raw[:], ps[:])
                            shuf = tpool.tile([P, 512], BF16, tag="t2s")
                            nc.vector.stream_shuffle(shuf[:], raw[:], SHUF_MASK)
                            t1 = tpool.tile([P, 512], BF16, tag="t1")
                            nc.vector.tensor_mul(
                                t1[:],
                                raw[:],
                                ck[:, src_col * 512 : (src_col + 1) * 512],
                            )
                            t2 = tpool.tile([P, 512], BF16, tag="t2")
                            nc.vector.tensor_mul(
                                t2[:],
                                shuf[:],
                                sk[:, src_col * 512 : (src_col + 1) * 512],
                            )
                            nc.vector.tensor_add(
                                dst[:, dst_col * 512 : (dst_col + 1) * 512],
                                t1[:],
                                t2[:],
                            )

                        return [ha, hb]

                    def emit_proj_jo(jo, preload=False):
                        """Q^T/K^T (fp8) for head pair jo; returns 12 half-step
                        closures: q0a q0b q1a q1b k0a..k3b."""
                        ktp = kqpool.tile([P, T], F8, tag="ktp")
                        qtp = kqpool.tile([P, TQ + 512], F8, tag="qtp")

                        def load_w():
                            wq_t = wpool.tile([P, DO, P], BF16, tag="wq")
                            nc.gpsimd.dma_start(
                                wq_t[:], Wq_r[:, :, jo * P : (jo + 1) * P]
                            )
                            wk_t = wpool.tile([P, DO, P], BF16, tag="wk")
                            nc.gpsimd.dma_start(
                                wk_t[:], Wk_r[:, :, jo * P : (jo + 1) * P]
                            )
                            nc.vector.memset(qtp[:, TQ : TQ + 512], 0.0)
                            return wq_t, wk_t

                        wref = []
                        if preload:
                            wref.extend(load_w())

                        halves = []
                        for i in range(6):
                            ref = {}

                            def mk(i=i, ref=ref):
                                def lazy_a():
                                    if not wref:
                                        wref.extend(load_w())
                                    wq_t, wk_t = wref
                                    if i < 2:
                                        hs = proj_rope_halves(wq_t, qtp, i, i)
                                    else:
                                        hs = proj_rope_halves(
                                            wk_t, ktp, i - 2, i - 2
                                        )
                                    ref["hs"] = hs
                                    hs[0]()

                                def lazy_b():
                                    ref["hs"][1]()

                                return [lazy_a, lazy_b]

                            halves.extend(mk())
                        return ktp, qtp, halves

                    def load_wv(g):
                        wv_t = wvgpool.tile([P, DO, 256], BF16, tag="wvg")
                        nc.gpsimd.dma_start(
                            wv_t[:], Wv_r[:, :, g * 256 : (g + 1) * 256]
                        )
                        return wv_t

                    def v_step(wv_t, g, to):
                        """V projection for head group g (4 heads), key tile to."""
                        ps = pmisc.tile([P, 512], F32, tag="misc", name="psv")
                        for do in range(DO):
                            nc.tensor.matmul(
                                ps[:, 0:256],
                                xts[:, do, to * P : (to + 1) * P],
                                wv_t[:, do],
                                start=(do == 0),
                                stop=(do == DO - 1),
                            )
                        nc.vector.tensor_copy(
                            v8[:, to, 4 * g : 4 * g + 4, 0:64],
                            ps[:, 0:256].rearrange("tp (h e) -> tp h e", e=64),
                        )

                    # ------- phase A: V group 0 (to 0..7) + pair-0 Q/K proj ----
                    with ExitStack() as p1b:
                        pmmv = p1b.enter_context(
                            tc.tile_pool(name="ps_v", bufs=2, space="PSUM")
                        )
                        # PE warm-up: dummy matmuls on a memset tile keep the
                        # tensor engine busy through the initial DMA wait so
                        # the p-state ramp (0.65->2.4GHz over 3us of
                        # continuous execution) completes before real work.
                        wrm = tpool.tile([P, P], BF16, tag="raw", name="warm")
                        nc.vector.memset(wrm[:], 0.0)
                        wps = pmisc.tile([P, 512], F32, tag="misc", name="warmps")
                        for _ in range(12):
                            nc.tensor.matmul(
                                wps[:, 0:P], wrm[:], wrm[:], start=True, stop=True
                            )
                        nc.sync.dma_start(xts[:, :, 0:256], xT_r[:, :, 0:256])
                        wv_g0 = load_wv(0)
                        nc.sync.dma_start(
                            xts[:, :, 256:512], xT_r[:, :, 256:512]
                        )
                        nc.gpsimd.dma_start(
                            xts[:, :, 512:1024], xT_r[:, :, 512:1024]
                        )
                        ktp0, qtp0, halves0 = emit_proj_jo(0, preload=True)
                        nc.sync.dma_start(ck[:], cosk[:])
                        nc.sync.dma_start(sk[:], sink[:])
                        nc.gpsimd.dma_start(
                            xts[:, :, 1536:2048], xT_r[:, :, 1536:2048]
                        )
                        nc.sync.dma_start(
                            xts[:, :, 1024:1536], xT_r[:, :, 1024:1536]
                        )
                        nc.vector.memset(v8[:, :, :, 64:65], 1.0)

                        def v0_step(to):
                            ps = pmmv.tile([P, 512], F32, tag="psv")
                            for do in range(DO):
                                nc.tensor.matmul(
                                    ps[:, 0:256],
                                    xts[:, do, to * P : (to + 1) * P],
                                    wv_g0[:, do],
                                    start=(do == 0),
                                    stop=(do == DO - 1),
                                )
                            nc.vector.tensor_copy(
                                v8[:, to, 0:4, 0:64],
                                ps[:, 0:256].rearrange("tp (h e) -> tp h e", e=64),
                            )

                        # proj-0: q0 (halves 0,1) + k0..k3 (halves 4..11);
                        # q1 (halves 2,3) is deferred into attention(0).
                        pa_proj = [halves0[i] for i in (0, 1, 4, 5, 6, 7, 8, 9, 10, 11)]
                        # V tiles 0-3 run while the wq/wk loads land; the proj
                        # halves then interleave with the rest.
                        seq = [0, 1, 2, 3, "p0", "p1", "p2", "p3", 4, 5,
                               "p4", "p5", 6, 7, "p6", "p7", "p8", "p9"]
                        for item in seq:
                            if isinstance(item, int):
                                v0_step(item)
                            else:
                                pa_proj[int(item[1:])]()

                    apool = pmain.enter_context(tc.tile_pool(name="aotp", bufs=2))
                    smpool = pmain.enter_context(tc.tile_pool(name="sm", bufs=3))
                    pss = pmain.enter_context(
                        tc.tile_pool(name="ps_s", bufs=2, space="PSUM")
                    )
                    pav = pmain.enter_context(
                        tc.tile_pool(name="ps_av", bufs=2, space="PSUM")
                    )

                    def emit_attention(p, ktp, qtp, av_prev, tr_prev, work):
                        """Attention for head pair p.

                        av_prev: 16 AV-stream closures of pair p-1's second
                        query chunk -- run at this pair's lc0 units 0..7.
                        tr_prev: pair p-1's transpose closure (unit 8).
                        work: proj/V closures for upcoming pairs, spread over
                        units 8..15 of both chunks.
                        Returns (av_lc1_closures, tr_closure) for pair p.
                        """
                        q3 = qtp.rearrange("d (s l) -> d s l", s=3)
                        aotP = apool.tile([P, TQ // P, P], BF16, tag="aotP")

                        def av_stream(lc, h, lt):
                            ltg = lc * 4 + lt
                            pav_t = pav.tile([P, 512], F32, tag="pav")
                            for to in range(NT):
                                s = (p * 32 + lc * NT + to) % RING
                                nc.tensor.matmul(
                                    pav_t[:, 0:65],
                                    p8[
                                        :,
                                        s,
                                        512 * h + lt * P : 512 * h + (lt + 1) * P,
                                    ],
                                    v8[:, to, 2 * p + h, :],
                                    start=(to == 0),
                                    stop=(to == NT - 1),
                                )
                            avs = smpool.tile([P, 65], F32, tag="avs")
                            nc.vector.tensor_copy(avs[:], pav_t[:, 0:65])
                            rc = smpool.tile([P, 1], F32, tag="rc")
                            with nc.allow_low_precision(
                                reason="softmax denom reciprocal"
                            ):
                                nc.vector.reciprocal(rc[:], avs[:, 64:65])
                            nc.vector.tensor_scalar_mul(
                                aotP[:, ltg, 64 * h : 64 * h + 64],
                                avs[:, 0:64],
                                rc[:],
                            )

                        av_cl = {
                            lc: [
                                (lambda lc=lc, h=h, lt=lt: av_stream(lc, h, lt))
                                for h in range(2)
                                for lt in range(4)
                            ]
                            for lc in range(2)
                        }

                        def tr_closure():
                            for ltg in range(TQ // P):
                                nc.sync.dma_start_transpose(
                                    aotT[:, p, ltg * P : (ltg + 1) * P],
                                    aotP[:, ltg, :],
                                )

                        # slot schedule: unit -> closures
                        sched = {u: [] for u in range(32)}
                        if av_prev is not None:
                            for i, cl in enumerate(av_prev):
                                sched[i // 2].append(cl)
                        for i, cl in enumerate(av_cl[0]):
                            sched[16 + i // 2].append(cl)
                        if tr_prev is not None:
                            sched[8].append(tr_prev)
                        wslots = list(range(8, 16)) + list(range(24, 32))
                        if av_prev is None:
                            wslots = list(range(0, 16)) + list(range(24, 32))
                        wq = list(work)
                        ui = 0
                        while wq and ui < len(wslots):
                            u = wslots[ui]
                            item = wq[0]
                            if isinstance(item, tuple) and item[0] == "heavy":
                                if not sched[u]:
                                    sched[u].append(item[1])
                                    wq.pop(0)
                                    ui += 2
                                else:
                                    ui += 1
                                continue
                            if len(sched[u]) < 2:
                                sched[u].append(item)
                                wq.pop(0)
                            else:
                                ui += 1
                        assert not wq, f"work overflow pair {p}: {len(wq)} left"

                        for lc in range(2):
                            for to in range(NT):
                                s = (p * 32 + lc * NT + to) % RING
                                pst = pss.tile([P, 1024], F32, tag="pss")
                                for h in range(2):
                                    r0 = 64 * h
                                    lhsT = (
                                        ktp[r0 : r0 + 64, to * P : (to + 1) * P]
                                        .unsqueeze(1)
                                        .broadcast_to([64, 2, P])
                                    )
                                    if lc == 0:
                                        rhs = q3[r0 : r0 + 64, 0::2, :]
                                    else:
                                        rhs = q3[r0 : r0 + 64, 1:3, :]
                                    nc.tensor.matmul(
                                        pst[:, 512 * h : 512 * h + 512],
                                        lhsT,
                                        rhs,
                                        start=True,
                                        stop=True,
                                        perf_mode=DR,
                                    )
                                nc.scalar.activation(
                                    p8[:, s, :], pst[:], AF.Exp, scale=SCALE
                                )
                                for cl in sched[lc * NT + to]:
                                    cl()
                        return av_cl[1], tr_closure

                    # V group g (heads 4g..4g+3) is produced across attentions
                    # 2g-2 and 2g-1; its consumers are pairs 2g, 2g+1.
                    wv_ref = {}
                    ktp, qtp = ktp0, qtp0
                    av_prev = None
                    tr_prev = None
                    for p in range(DO):
                        work = []
                        if p == 0:
                            work += [halves0[2], halves0[3]]  # q1 of pair 0
                            work += [
                                (lambda to=to: v_step(wv_g0, 0, to))
                                for to in range(8, NT)
                            ]
                        if p >= 1:
                            g = (p + 1) // 2
                            if g <= 3:
                                if p % 2 == 1:
                                    wv_ref[g] = load_wv(g)
                                    rng_to = range(0, 8)
                                else:
                                    rng_to = range(8, NT)
                                wv_t = wv_ref[g]
                                work += [
                                    (lambda to=to, wv_t=wv_t, g=g: v_step(
                                        wv_t, g, to
                                    ))
                                    for to in rng_to
                                ]
                        if p + 1 < DO:
                            nktp, nqtp, nhalves = emit_proj_jo(p + 1)
                            work += nhalves
                        else:
                            nktp = nqtp = None
                        if p == 7:
                            # xts is dead after pair-7's projections; reuse its
                            # SBUF as the Wp/bias buffer so the output
                            # projection starts without DMA exposure
                            for ko in range(DO):
                                nc.sync.dma_start(
                                    xts[:, ko, 0:1024], Wp_r[:, ko]
                                )
                            xf = xts.bitcast(F32)
                            nc.sync.dma_start(xf[:, 0, 512:1024], bpb[:, 0:512])
                            nc.sync.dma_start(
                                xf[:, 1, 512:1024], bpb[:, 512:1024]
                            )
                        av_prev, tr_prev = emit_attention(
                            p, ktp, qtp, av_prev, tr_prev, work
                        )
                        ktp, qtp = nktp, nqtp
                    # tail: pair 7 second-chunk AV + its transposes
                    for cl in av_prev:
                        cl()
                    tr_prev()

                # ------- output projection tail (Wp lives in xts) ----------
                with ExitStack() as p3:
                    outpool = p3.enter_context(tc.tile_pool(name="outp", bufs=3))
                    ps_f = p3.enter_context(
                        tc.tile_pool(name="ps_f", bufs=4, space="PSUM")
                    )
                    xfv = xts.bitcast(F32)
                    for idx in range(16):
                        lt, jc = idx // 2, idx % 2
                        ps = ps_f.tile([P, 512], F32, tag="psf")
                        for ko in range(DO):
                            nc.tensor.matmul(
                                ps[:],
                                aotT[:, ko, lt * P : (lt + 1) * P],
                                xts[:, ko, jc * 512 : (jc + 1) * 512],
                                start=(ko == 0),
                                stop=(ko == DO - 1),
                            )
                        ot = outpool.tile([P, 512], F32, tag="oto")
                        nc.vector.tensor_add(
                            ot[:], ps[:], xfv[:, jc, 512:1024]
                        )
                        nc.sync.dma_start(
                            out[lt * P : (lt + 1) * P, jc * 512 : (jc + 1) * 512],
                            ot[:],
                        )

    if split_waits:
        _split_multi_waits(nc)
    return nc


def _rope_tables():
    inv = 1.0 / (ROPE_THETA ** (np.arange(0, HD, 2, dtype=np.float32) / HD))
    t = np.arange(T, dtype=np.float32)
    freqs = np.einsum("i,j->ij", t, inv)  # [T, 32]
    freqs = np.repeat(freqs, 2, axis=-1)  # [T, 64]
    cosT = np.cos(freqs).T  # [64, T]
    sinT = np.sin(freqs).T
    cosk = np.tile(cosT, (2, 1)).astype(np.float32)  # [128, T]
    sink = np.tile(sinT, (2, 1)).astype(np.float32)
    # sign-fold for the stream_shuffle rotate-half: rot(x)[2i] = -x[2i+1]
    sink[0::2] *= -1.0
    return np.ascontiguousarray(cosk), np.ascontiguousarray(sink)


_NC_CACHE = {}


def make_in_maps(x, Wq, Wk, Wv, Wp, bp):
    cosk, sink = _rope_tables()
    bpb = np.ascontiguousarray(np.tile(np.asarray(bp, np.float32)[None, :], (P, 1)))
    bf = lambda a: np.ascontiguousarray(np.asarray(a, np.float32)).astype(
        ml_dtypes.bfloat16
    )
    Wqb, Wkb, Wvb, Wpb = bf(Wq), bf(Wk), bf(Wv), bf(Wp)
    in_maps = []
    for c in range(8):
        b, qh = c // 2, c % 2
        xT = np.asarray(x[b], np.float32).T  # [D, T]
        roll = qh * TQ
        in_maps.append(
            {
                "xT": bf(np.roll(xT, -roll, axis=1)),
                "Wq": Wqb,
                "Wk": Wkb,
                "Wv": Wvb,
                "Wp": Wpb,
                "bpb": bpb,
                "cosk": bf(np.roll(cosk, -roll, axis=1)),
                "sink": bf(np.roll(sink, -roll, axis=1)),
            }
        )
    return in_maps


def kernel(x, h, w, Wq, Wk, Wv, Wp, bp, _trace=False, **trace_kwargs):
    x = np.asarray(x, np.float32)
    in_maps = make_in_maps(x, Wq, Wk, Wv, Wp, bp)
    if "nc" not in _NC_CACHE:
        _NC_CACHE["nc"] = build_nc()
    nc = _NC_CACHE["nc"]
    res = run_bass_kernel_spmd(
        nc, in_maps, list(range(8)), trace=_trace, **trace_kwargs
    )
    out = np.empty((B, T, D), np.float32)
    for c in range(8):
        b, qh = c // 2, c % 2
        out[b, qh * TQ : (qh + 1) * TQ, :] = res.results[c]["out"]
    kernel.last_result = res
    return out


# revision 50
# speedup vs baseline: 1.5535x; 1.0286x over previous
"""AttentionWithRoPE on 8 Trainium2 NeuronCores.

Sharding: batch x query-half -> 8 independent cores (no collectives).
Core c handles batch b=c//2, query rows [qh*1024, (qh+1)*1024) with qh=c%2.
The host rolls the t axis per core so the query rows always sit in columns
[0, TQ) of xT; cosk/sink are rolled identically, so RoPE sees true positions
and the softmax key set is unchanged (order-invariant).

Per-core plan (engine-balanced around the ACT exp street):
  Projections in bf16 (x, Wq/Wk/Wv/Wp host-converted; f32 PSUM accum).
  RoPE: signed pair-swap permutation matmul on PE + DVE combine; Q^T/K^T
  are written as fp8e4 (total quantization noise ~1% on the final
  output, well under the 2e-2 gate).
  S^T per head: ONE fp8 DoubleRow matmul at 0.5 cycles/row -- the d=64
  contraction rides slot 0 (lhsT = K^T with a stride-0 slot broadcast)
  while rhs slot 1 points at a zero strip appended to qtp, so no
  layout fold is needed.
  exp on ACT in [128, 1024] tiles (both heads of a pair share one
  2-bank PSUM tile) -> bf16 P^T ring buffer (RING slots; exp of chunk
  lc overlaps the AV consumption of chunk lc-1).
  AV with l on partitions: out[l, d|1] = P^T-tile.T @ [V|1]-tile, a
  65-column moving operand (65 rows/matmul), accumulated over the 16
  key tiles; softmax row sums ride along as column 64, normalized by
  DVE reciprocal + per-partition tensor_scalar multiply.
  attnout goes bf16 [l, j] -> [j, l] via DMA-engine transposes
  (dma_start_transpose), then out = aotT.T @ Wp + bp in bf16.
  V stays SBUF-resident in bf16 (no DRAM staging); attnout likewise.

Schedule: one "exp street" of 32 units per pair paced by ACT
(~1.04us/unit). Each unit emits S matmuls + the exp; interleaved
closures keep the PE busy: AV streams of the previous query chunk at
units 0..7, the previous pair's transposes at unit 8, and the next
pair's Q/K projection halves plus V-projection tiles at units 8..15 /
24..31. V group g (4 heads) is produced across attentions 2g-1 and 2g,
one half per pair, so every pair carries a near-equal PE load (~32us)
just under the ACT street (~33us).
"""

import sys

sys.path.insert(0, "/opt/trn_rl_repo")

from contextlib import ExitStack

import numpy as np
import ml_dtypes

import concourse.bass as bass
import concourse.mybir as mybir
import concourse.tile as tile
from concourse.bass_utils import run_bass_kernel_spmd

F32 = mybir.dt.float32
F32R = mybir.dt.float32r
BF16 = mybir.dt.bfloat16
F8 = mybir.dt.float8e4
AF = mybir.ActivationFunctionType
MUL = mybir.AluOpType.mult
DR = mybir.MatmulPerfMode.DoubleRow

B, T, D = 4, 2048, 1024
H, HD = 16, 64
P = 128
TQ = 1024  # query rows per core
NT = T // P  # 16 key tiles
DO = D // P  # 8 contraction tiles
RING = 26  # P^T ring slots (16 live + AV-closure lag, see emit_attention)
SCALE = float(D) ** -0.5
ROPE_THETA = 10000.0
SHUF_MASK = [i ^ 1 for i in range(32)]  # adjacent-partition swap (per 32-lane group)

_ws_ctr = [0]


def _split_multi_waits(nc):
    """The walrus build in this container accepts at most one sync-wait per
    engine instruction. Hoist all but one wait of each instruction into
    standalone EventSemaphore instructions on the same engine, placed
    immediately before it (engines are in-order, so semantics are identical)."""
    n = 0
    for f in nc.m.functions:
        for blk in f.blocks:
            insts = list(blk.instructions)
            newlist = []
            changed = False
            for inst in insts:
                si = getattr(inst, "sync_info", None)
                waits = list(si.on_wait) if si is not None and si.on_wait else []
                if len(waits) > 1:
                    for w in waits[:-1]:
                        _ws_ctr[0] += 1
                        evs = mybir.InstEventSemaphore(
                            name=f"WSPLIT-{_ws_ctr[0]}", ins=[], outs=[]
                        )
                        evs.engine = inst.engine
                        evs.sync_info = mybir.SyncInfo(on_wait=[w], on_update=[])
                        newlist.append(evs)
                        n += 1
                    inst.sync_info = mybir.SyncInfo(
                        on_wait=[waits[-1]], on_update=list(si.on_update)
                    )
                    changed = True
                newlist.append(inst)
            if changed:
                blk.instructions[:] = newlist
    return n


def build_nc(split_waits=True, reps=1):
    nc = bass.Bass(trn_type="TRN2", target_bir_lowering=False, debug=False)

    xT = nc.dram_tensor("xT", [D, T], BF16, kind="ExternalInput").ap()
    Wq = nc.dram_tensor("Wq", [D, D], BF16, kind="ExternalInput").ap()
    Wk = nc.dram_tensor("Wk", [D, D], BF16, kind="ExternalInput").ap()
    Wv = nc.dram_tensor("Wv", [D, D], BF16, kind="ExternalInput").ap()
    Wp = nc.dram_tensor("Wp", [D, D], BF16, kind="ExternalInput").ap()
    bpb = nc.dram_tensor("bpb", [P, D], F32, kind="ExternalInput").ap()
    cosk = nc.dram_tensor("cosk", [P, T], BF16, kind="ExternalInput").ap()
    sink = nc.dram_tensor("sink", [P, T], BF16, kind="ExternalInput").ap()
    out = nc.dram_tensor("out", [TQ, D], F32, kind="ExternalOutput").ap()

    xT_r = xT.rearrange("(do dp) t -> dp do t", dp=P)  # [128, 8, 2048]
    Wq_r = Wq.rearrange("(do dp) j -> dp do j", dp=P)
    Wk_r = Wk.rearrange("(do dp) j -> dp do j", dp=P)
    Wv_r = Wv.rearrange("(do dp) j -> dp do j", dp=P)
    Wp_r = Wp.rearrange("(ko kp) j -> kp ko j", kp=P)

    with tile.TileContext(nc) as tc:
      for _rep in range(reps):
        with ExitStack() as top:
            persist = top.enter_context(tc.tile_pool(name="persist", bufs=1))
            ck = persist.tile([P, T], BF16)
            sk = persist.tile([P, T], BF16)
            v8 = persist.tile([P, NT, H, 65], BF16)
            p8 = persist.tile([P, RING, 1024], BF16)
            aotT = persist.tile([P, DO, TQ], BF16)

            with tc.tile_pool(name="xpool", bufs=1) as xpool:
                xts = xpool.tile([P, DO, T], BF16)

                with ExitStack() as pmain:
                    wvgpool = pmain.enter_context(tc.tile_pool(name="w_vg", bufs=2))
                    wpool = pmain.enter_context(tc.tile_pool(name="w_qk", bufs=2))
                    tpool = pmain.enter_context(tc.tile_pool(name="t_qk", bufs=2))
                    kqpool = pmain.enter_context(tc.tile_pool(name="kq", bufs=2))
                    pmm = pmain.enter_context(
                        tc.tile_pool(name="ps_qk", bufs=1, space="PSUM")
                    )
                    pmisc = pmain.enter_context(
                        tc.tile_pool(name="ps_misc", bufs=1, space="PSUM")
                    )

                    def proj_rope_halves(w_t, dst, dst_col, src_col):
                        """One [128, 512] Q/K projection tile + RoPE into dst
                        (fp8), split into two closures so the PE burst
                        interleaves finely with the exp street."""
                        ref = {}

                        def ha():
                            ps = pmm.tile([P, 512], F32, tag="ps")
                            ref["ps"] = ps
                            for do in range(4):
                                nc.tensor.matmul(
                                    ps[:],
                                    w_t[:, do],
                                    xts[:, do, src_col * 512 : (src_col + 1) * 512],
                                    start=(do == 0),
                                    stop=False,
                                )

                        def hb():
                            ps = ref["ps"]
                            for do in range(4, DO):
                                nc.tensor.matmul(
                                    ps[:],
                                    w_t[:, do],
                                    xts[:, do, src_col * 512 : (src_col + 1) * 512],
                                    start=False,
                                    stop=(do == DO - 1),
                                )
                            raw = tpool.tile([P, 512], BF16, tag="raw")
                            nc.vector.tensor_copy(raw[:], ps[:])
                            shuf = tpool.tile([P, 512], BF16, tag="t2s")
                            nc.vector.stream_shuffle(shuf[:], raw[:], SHUF_MASK)
                            t1 = tpool.tile([P, 512], BF16, tag="t1")
                            nc.vector.tensor_mul(
                                t1[:],
                                raw[:],
                                ck[:, src_col * 512 : (src_col + 1) * 512],
                            )
                            t2 = tpool.tile([P, 512], BF16, tag="t2")
                            nc.vector.tensor_mul(
                                t2[:],
                                shuf[:],
                                sk[:, src_col * 512 : (src_col + 1) * 512],
                            )
                            nc.vector.tensor_add(
                                dst[:, dst_col * 512 : (dst_col + 1) * 512],
                                t1[:],
                                t2[:],
                            )

                        return [ha, hb]

                    def emit_proj_jo(jo, preload=False):
                        """Q^T/K^T (fp8) for head pair jo; returns 12 half-step
                        closures: q0a q0b q1a q1b k0a..k3b."""
                        ktp = kqpool.tile([P, T], F8, tag="ktp")
                        qtp = kqpool.tile([P, TQ + 512], F8, tag="qtp")

                        def load_w():
                            wq_t = wpool.tile([P, DO, P], BF16, tag="wq")
                            nc.gpsimd.dma_start(
                                wq_t[:], Wq_r[:, :, jo * P : (jo + 1) * P]
                            )
                            wk_t = wpool.tile([P, DO, P], BF16, tag="wk")
                            nc.gpsimd.dma_start(
                                wk_t[:], Wk_r[:, :, jo * P : (jo + 1) * P]
                            )
                            nc.vector.memset(qtp[:, TQ : TQ + 512], 0.0)
                            return wq_t, wk_t

                        wref = []
                        if preload:
                            wref.extend(load_w())

                        halves = []
                        for i in range(6):
                            ref = {}

                            def mk(i=i, ref=ref):
                                def lazy_a():
                                    if not wref:
                                        wref.extend(load_w())
                                    wq_t, wk_t = wref
                                    if i < 2:
                                        hs = proj_rope_halves(wq_t, qtp, i, i)
                                    else:
                                        hs = proj_rope_halves(
                                            wk_t, ktp, i - 2, i - 2
                                        )
                                    ref["hs"] = hs
                                    hs[0]()

                                def lazy_b():
                                    ref["hs"][1]()

                                return [lazy_a, lazy_b]

                            halves.extend(mk())
                        return ktp, qtp, halves

                    def load_wv(g):
                        wv_t = wvgpool.tile([P, DO, 256], BF16, tag="wvg")
                        nc.gpsimd.dma_start(
                            wv_t[:], Wv_r[:, :, g * 256 : (g + 1) * 256]
                        )
                        return wv_t

                    def v_step(wv_t, g, to):
                        """V projection for head group g (4 heads), key tile to."""
                        ps = pmisc.tile([P, 512], F32, tag="misc", name="psv")
                        for do in range(DO):
                            nc.tensor.matmul(
                                ps[:, 0:256],
                                xts[:, do, to * P : (to + 1) * P],
                                wv_t[:, do],
                                start=(do == 0),
                                stop=(do == DO - 1),
                            )
                        nc.vector.tensor_copy(
                            v8[:, to, 4 * g : 4 * g + 4, 0:64],
                            ps[:, 0:256].rearrange("tp (h e) -> tp h e", e=64),
                        )

                    # ------- phase A: V group 0 (to 0..7) + pair-0 Q/K proj ----
                    with ExitStack() as p1b:
                        pmmv = p1b.enter_context(
                            tc.tile_pool(name="ps_v", bufs=2, space="PSUM")
                        )
                        # PE warm-up: dummy matmuls on a memset tile keep the
                        # tensor engine busy through the initial DMA wait so
                        # the p-state ramp (0.65->2.4GHz over 3us of
                        # continuous execution) completes before real work.
                        wrm = tpool.tile([P, P], BF16, tag="raw", name="warm")
                        nc.vector.memset(wrm[:], 0.0)
                        wps = pmisc.tile([P, 512], F32, tag="misc", name="warmps")
                        for _ in range(12):
                            nc.tensor.matmul(
                                wps[:, 0:P], wrm[:], wrm[:], start=True, stop=True
                            )
                        nc.sync.dma_start(xts[:, :, 0:256], xT_r[:, :, 0:256])
                        wv_g0 = load_wv(0)
                        nc.sync.dma_start(
                            xts[:, :, 256:512], xT_r[:, :, 256:512]
                        )
                        nc.gpsimd.dma_start(
                            xts[:, :, 512:1024], xT_r[:, :, 512:1024]
                        )
                        ktp0, qtp0, halves0 = emit_proj_jo(0, preload=True)
                        nc.sync.dma_start(ck[:], cosk[:])
                        nc.sync.dma_start(sk[:], sink[:])
                        nc.gpsimd.dma_start(
                            xts[:, :, 1536:2048], xT_r[:, :, 1536:2048]
                        )
                        nc.sync.dma_start(
                            xts[:, :, 1024:1536], xT_r[:, :, 1024:1536]
                        )
                        nc.vector.memset(v8[:, :, :, 64:65], 1.0)

                        def v0_step(to):
                            ps = pmmv.tile([P, 512], F32, tag="psv")
                            for do in range(DO):
                                nc.tensor.matmul(
                                    ps[:, 0:256],
                                    xts[:, do, to * P : (to + 1) * P],
                                    wv_g0[:, do],
                                    start=(do == 0),
                                    stop=(do == DO - 1),
                                )
                            nc.vector.tensor_copy(
                                v8[:, to, 0:4, 0:64],
                                ps[:, 0:256].rearrange("tp (h e) -> tp h e", e=64),
                            )

                        # proj-0: q0 (halves 0,1) + k0..k3 (halves 4..11);
                        # q1 (halves 2,3) is deferred into attention(0).
                        pa_proj = [halves0[i] for i in (0, 1, 4, 5, 6, 7, 8, 9, 10, 11)]
                        # V tiles 0-3 run while the wq/wk loads land; the proj
                        # halves then interleave with the rest.
                        seq = [0, 1, 2, 3, "p0", "p1", "p2", "p3", 4, 5,
                               "p4", "p5", 6, 7, "p6", "p7", "p8", "p9"]
                        for item in seq:
                            if isinstance(item, int):
                                v0_step(item)
                            else:
                                pa_proj[int(item[1:])]()

                    apool = pmain.enter_context(tc.tile_pool(name="aotp", bufs=2))
                    smpool = pmain.enter_context(tc.tile_pool(name="sm", bufs=3))
                    pss = pmain.enter_context(
                        tc.tile_pool(name="ps_s", bufs=2, space="PSUM")
                    )
                    pav = pmain.enter_context(
                        tc.tile_pool(name="ps_av", bufs=2, space="PSUM")
                    )

                    def emit_attention(
                        p, ktp, qtp, av_prev, tr_prev, work, late=None
                    ):
                        """Attention for head pair p.

                        av_prev: 16 AV-stream closures of pair p-1's second
                        query chunk -- run at this pair's lc0 units 0..7.
                        tr_prev: pair p-1's transpose closure (unit 8).
                        work: proj/V closures for upcoming pairs, spread over
                        units 8..15 of both chunks.
                        Returns (av_lc1_closures, tr_closure) for pair p.
                        """
                        q3 = qtp.rearrange("d (s l) -> d s l", s=3)
                        aotP = apool.tile([P, TQ // P, P], BF16, tag="aotP")

                        def av_stream(lc, h, lt):
                            ltg = lc * 4 + lt
                            pav_t = pav.tile([P, 512], F32, tag="pav")
                            for to in range(NT):
                                s = (p * 32 + lc * NT + to) % RING
                                nc.tensor.matmul(
                                    pav_t[:, 0:65],
                                    p8[
                                        :,
                                        s,
                                        512 * h + lt * P : 512 * h + (lt + 1) * P,
                                    ],
                                    v8[:, to, 2 * p + h, :],
                                    start=(to == 0),
                                    stop=(to == NT - 1),
                                )
                            avs = smpool.tile([P, 65], F32, tag="avs")
                            nc.vector.tensor_copy(avs[:], pav_t[:, 0:65])
                            rc = smpool.tile([P, 1], F32, tag="rc")
                            with nc.allow_low_precision(
                                reason="softmax denom reciprocal"
                            ):
                                nc.vector.reciprocal(rc[:], avs[:, 64:65])
                            nc.vector.tensor_scalar_mul(
                                aotP[:, ltg, 64 * h : 64 * h + 64],
                                avs[:, 0:64],
                                rc[:],
                            )

                        av_cl = {
                            lc: [
                                (lambda lc=lc, h=h, lt=lt: av_stream(lc, h, lt))
                                for h in range(2)
                                for lt in range(4)
                            ]
                            for lc in range(2)
                        }

                        def tr_closure(rng_ltg=range(TQ // P)):
                            for ltg in rng_ltg:
                                nc.sync.dma_start_transpose(
                                    aotT[:, p, ltg * P : (ltg + 1) * P],
                                    aotP[:, ltg, :],
                                )

                        # slot schedule: unit -> closures
                        sched = {u: [] for u in range(32)}
                        if av_prev is not None:
                            for i, cl in enumerate(av_prev):
                                sched[i // 2].append(cl)
                        for i, cl in enumerate(av_cl[0]):
                            sched[16 + i // 2].append(cl)
                        if tr_prev is not None:
                            sched[8].append(tr_prev)
                        wslots = list(range(8, 16)) + list(range(24, 32))
                        if av_prev is None:
                            wslots = list(range(0, 16)) + list(range(24, 32))
                        if late is not None:
                            lpos = [20, 21, 22, 24, 25, 26, 28, 29, 30]
                            for i, cl in enumerate(late(aotP, tr_closure)):
                                sched[lpos[i]].append(cl)
                        wq = list(work)
                        ui = 0
                        while wq and ui < len(wslots):
                            u = wslots[ui]
                            item = wq[0]
                            if isinstance(item, tuple) and item[0] == "heavy":
                                if not sched[u]:
                                    sched[u].append(item[1])
                                    wq.pop(0)
                                    ui += 2
                                else:
                                    ui += 1
                                continue
                            if len(sched[u]) < 2:
                                sched[u].append(item)
                                wq.pop(0)
                            else:
                                ui += 1
                        assert not wq, f"work overflow pair {p}: {len(wq)} left"

                        for lc in range(2):
                            for to in range(NT):
                                s = (p * 32 + lc * NT + to) % RING
                                pst = pss.tile([P, 1024], F32, tag="pss")
                                for h in range(2):
                                    r0 = 64 * h
                                    lhsT = (
                                        ktp[r0 : r0 + 64, to * P : (to + 1) * P]
                                        .unsqueeze(1)
                                        .broadcast_to([64, 2, P])
                                    )
                                    if lc == 0:
                                        rhs = q3[r0 : r0 + 64, 0::2, :]
                                    else:
                                        rhs = q3[r0 : r0 + 64, 1:3, :]
                                    nc.tensor.matmul(
                                        pst[:, 512 * h : 512 * h + 512],
                                        lhsT,
                                        rhs,
                                        start=True,
                                        stop=True,
                                        perf_mode=DR,
                                    )
                                nc.scalar.activation(
                                    p8[:, s, :], pst[:], AF.Exp, scale=SCALE
                                )
                                for cl in sched[lc * NT + to]:
                                    cl()
                        return av_cl[1], tr_closure

                    # V group g (heads 4g..4g+3) is produced across attentions
                    # 2g-2 and 2g-1; its consumers are pairs 2g, 2g+1.
                    wv_ref = {}
                    ktp, qtp = ktp0, qtp0
                    av_prev = None
                    tr_prev = None
                    for p in range(DO):
                        work = []
                        if p == 0:
                            work += [halves0[2], halves0[3]]  # q1 of pair 0
                            work += [
                                (lambda to=to: v_step(wv_g0, 0, to))
                                for to in range(8, NT)
                            ]
                        if p >= 1:
                            g = (p + 1) // 2
                            if g <= 3:
                                if p % 2 == 1:
                                    wv_ref[g] = load_wv(g)
                                    rng_to = range(0, 8)
                                else:
                                    rng_to = range(8, NT)
                                wv_t = wv_ref[g]
                                work += [
                                    (lambda to=to, wv_t=wv_t, g=g: v_step(
                                        wv_t, g, to
                                    ))
                                    for to in rng_to
                                ]
                        if p + 1 < DO:
                            nktp, nqtp, nhalves = emit_proj_jo(p + 1)
                            work += nhalves
                        else:
                            nktp = nqtp = None
                        if p == 7:
                            # xts is dead after pair-7's projections; reuse its
                            # SBUF as the Wp/bias buffer so the output
                            # projection starts without DMA exposure
                            for ko in range(DO):
                                nc.sync.dma_start(
                                    xts[:, ko, 0:1024], Wp_r[:, ko]
                                )
                            xf = xts.bitcast(F32)
                            nc.sync.dma_start(xf[:, 0, 512:1024], bpb[:, 0:512])
                            nc.sync.dma_start(
                                xf[:, 1, 512:1024], bpb[:, 512:1024]
                            )
                        late = None
                        if p == 7:
                            def late7(aotP7, tr_cl):
                                xfv7 = xts.bitcast(F32)

                                def op_tile(idx):
                                    lt, jc = idx // 2, idx % 2
                                    pool, tg = (
                                        (pmm, "ps") if idx % 2 == 0
                                        else (pmisc, "misc")
                                    )
                                    ps = pool.tile(
                                        [P, 512], F32, tag=tg, name="opearly"
                                    )
                                    for ko in range(DO):
                                        nc.tensor.matmul(
                                            ps[:],
                                            aotT[:, ko, lt * P : (lt + 1) * P],
                                            xts[
                                                :, ko,
                                                jc * 512 : (jc + 1) * 512,
                                            ],
                                            start=(ko == 0),
                                            stop=(ko == DO - 1),
                                        )
                                    ot = smpool.tile([P, 512], F32, tag="oto")
                                    nc.vector.tensor_add(
                                        ot[:], ps[:], xfv7[:, jc, 512:1024]
                                    )
                                    nc.sync.dma_start(
                                        out[
                                            lt * P : (lt + 1) * P,
                                            jc * 512 : (jc + 1) * 512,
                                        ],
                                        ot[:],
                                    )

                                cls = [lambda: tr_cl(range(0, 4))]
                                cls += [
                                    (lambda i=i: op_tile(i)) for i in range(8)
                                ]
                                return cls

                            late = late7
                        av_prev, tr_prev = emit_attention(
                            p, ktp, qtp, av_prev, tr_prev, work, late=late
                        )
                        ktp, qtp = nktp, nqtp
                    # tail: pair 7 second-chunk AV + its remaining transposes
                    for cl in av_prev:
                        cl()
                    tr_prev(range(4, TQ // P))

                # ------- output projection tail (Wp lives in xts) ----------
                with ExitStack() as p3:
                    outpool = p3.enter_context(tc.tile_pool(name="outp", bufs=3))
                    ps_f = p3.enter_context(
                        tc.tile_pool(name="ps_f", bufs=4, space="PSUM")
                    )
                    xfv = xts.bitcast(F32)
                    for idx in range(8, 16):
                        lt, jc = idx // 2, idx % 2
                        ps = ps_f.tile([P, 512], F32, tag="psf")
                        for ko in range(DO):
                            nc.tensor.matmul(
                                ps[:],
                                aotT[:, ko, lt * P : (lt + 1) * P],
                                xts[:, ko, jc * 512 : (jc + 1) * 512],
                                start=(ko == 0),
                                stop=(ko == DO - 1),
                            )
                        ot = outpool.tile([P, 512], F32, tag="oto")
                        nc.vector.tensor_add(
                            ot[:], ps[:], xfv[:, jc, 512:1024]
                        )
                        nc.sync.dma_start(
                            out[lt * P : (lt + 1) * P, jc * 512 : (jc + 1) * 512],
                            ot[:],
                        )

    if split_waits:
        _split_multi_waits(nc)
    return nc


def _rope_tables():
    inv = 1.0 / (ROPE_THETA ** (np.arange(0, HD, 2, dtype=np.float32) / HD))
    t = np.arange(T, dtype=np.float32)
    freqs = np.einsum("i,j->ij", t, inv)  # [T, 32]
    freqs = np.repeat(freqs, 2, axis=-1)  # [T, 64]
    cosT = np.cos(freqs).T  # [64, T]
    sinT = np.sin(freqs).T
    cosk = np.tile(cosT, (2, 1)).astype(np.float32)  # [128, T]
    sink = np.tile(sinT, (2, 1)).astype(np.float32)
    # sign-fold for the stream_shuffle rotate-half: rot(x)[2i] = -x[2i+1]
    sink[0::2] *= -1.0
    return np.ascontiguousarray(cosk), np.ascontiguousarray(sink)


_NC_CACHE = {}


def make_in_maps(x, Wq, Wk, Wv, Wp, bp):
    cosk, sink = _rope_tables()
    bpb = np.ascontiguousarray(np.tile(np.asarray(bp, np.float32)[None, :], (P, 1)))
    bf = lambda a: np.ascontiguousarray(np.asarray(a, np.float32)).astype(
        ml_dtypes.bfloat16
    )
    Wqb, Wkb, Wvb, Wpb = bf(Wq), bf(Wk), bf(Wv), bf(Wp)
    in_maps = []
    for c in range(8):
        b, qh = c // 2, c % 2
        xT = np.asarray(x[b], np.float32).T  # [D, T]
        roll = qh * TQ
        in_maps.append(
            {
                "xT": bf(np.roll(xT, -roll, axis=1)),
                "Wq": Wqb,
                "Wk": Wkb,
                "Wv": Wvb,
                "Wp": Wpb,
                "bpb": bpb,
                "cosk": bf(np.roll(cosk, -roll, axis=1)),
                "sink": bf(np.roll(sink, -roll, axis=1)),
            }
        )
    return in_maps


def kernel(x, h, w, Wq, Wk, Wv, Wp, bp, _trace=False, **trace_kwargs):
    x = np.asarray(x, np.float32)
    in_maps = make_in_maps(x, Wq, Wk, Wv, Wp, bp)
    if "nc" not in _NC_CACHE:
        _NC_CACHE["nc"] = build_nc()
    nc = _NC_CACHE["nc"]
    res = run_bass_kernel_spmd(
        nc, in_maps, list(range(8)), trace=_trace, **trace_kwargs
    )
    out = np.empty((B, T, D), np.float32)
    for c in range(8):
        b, qh = c // 2, c % 2
        out[b, qh * TQ : (qh + 1) * TQ, :] = res.results[c]["out"]
    kernel.last_result = res
    return out
